# revision 26
# baseline (speedup 1.0000x reference)
"""AttnBlock fusion kernel for Trainium2 (Bass/Tile), 8 NeuronCores.

Reference computation (per batch element b; c=512 channels, hw=1024 spatial):
    h  = GroupNorm(32, c)(x) ; k = Wk h + bk ; v = Wv h + bv
    y_ = GroupNorm(32, c)(y) ; q = Wq y_ + bq
    attn = softmax_j(q^T k / sqrt(c)) ; o = v @ attn^T ; out = x + Wp o + bp

Sharding: pure data parallel over batch (16 batches / 8 cores = 2 each).

Key algebraic moves (exactness notes inline):
  * bk cancels in softmax over j (adds a per-i constant to logits) -> dropped.
  * bv contributes bv * sum_j(attn) = bv exactly -> folded into
    bp' = bp + Wp @ bv on the host.
  * v is produced directly transposed (vT[j,c]) by the projection matmul,
    and attention scores are computed as S[j,i]; no on-chip transposes.
  * softmax denominator: ones matmul gives the partition-broadcast
    column sum of exp(S) directly in PSUM.

Precision strategy (tolerance is rel_l2 < 2e-2; this lands ~7e-3):
  * x/y stream in as bf16 (halves input DMA); GroupNorm statistics and all
    PSUM accumulation stay fp32.
  * All matmuls run in fp8 e4m3 with perf_mode=DoubleRow (2 fp8 weights
    per PE cell -> 2 contraction rows/instruction). Operands are laid out
    [P, ksub, free] so a pair of 128-row k-subtiles feeds each instruction.
  * exp uses bias=-3 so e^(logit-3) stays far below the e4m3 +-240
    limit (TRN fp8e4 overflows to inf, which would poison the matmul).
    The -3 cancels exactly in softmax normalization.
  * Output is written bf16 and upcast on host.

Engine budget notes: DVE/ACT instructions cost ~0.3us fixed overhead each,
so PSUM results are paired into [P, 2, 512] tiles (two banks) and
evacuated with a single op. GroupNorm statistics post-processing runs
entirely on DVE (bit-trick rsqrt) so the ACT table never leaves Exp
during attention; GroupNorm application runs on GpSimd (SBUF-only).
"""

import math
import os
import sys

import numpy as np

for _p in ("/opt/trn_rl_repo", "/root/.axon_site/_ro/trn_rl_repo"):
    if os.path.isdir(_p) and _p not in sys.path:
        sys.path.append(_p)

import ml_dtypes

import concourse.bass as bass
import concourse.bacc as bacc
import concourse.mybir as mybir
import concourse.tile as tile
from concourse.bass_utils import run_bass_kernel_spmd

F32 = mybir.dt.float32
I32 = mybir.dt.int32
BF16 = mybir.dt.bfloat16
F8 = mybir.dt.float8e4
AF = mybir.ActivationFunctionType
ALU = mybir.AluOpType
DR = mybir.MatmulPerfMode.DoubleRow

B, C, H, W = 16, 512, 32, 32
HW = H * W                  # 1024
NCORES = 8
BPC = B // NCORES           # 2 batches per core
P = 128                     # SBUF partitions
CT = C // P                 # 4 channel tiles
JT = HW // P                # 8 key-position tiles
IBS = 512                   # i-block size (query positions per block)
IB = HW // IBS              # 2 i-blocks
GROUPS = 32
GSIZE = C // GROUPS         # 16 channels per group
EPS = 1e-6
SM_SCALE = float(int(C) ** -0.5)
EXP_BIAS = -3.0             # keeps exp() outputs well inside e4m3 range
MAGIC = 0x5F3759DF          # fp32 rsqrt seed

# prm rows: gn scales (x,y), gn biases (x,y), bq, bp'
R_SCALE, R_BIAS, R_BQ, R_BPP = 0, 2, 4, 5


def _emit(tc, aps):
    nc = tc.nc
    xs, ys, wq, wk, wv, wp, prm, amat, out = (
        aps["xs"], aps["ys"], aps["wqT"], aps["wkT"], aps["wvT"], aps["wpT"],
        aps["prm"], aps["amat"], aps["out"],
    )

    from contextlib import ExitStack

    with ExitStack() as ctx:
        cpool = ctx.enter_context(tc.tile_pool(name="const", bufs=1))
        wpool = ctx.enter_context(tc.tile_pool(name="w", bufs=1))
        xpool = ctx.enter_context(tc.tile_pool(name="xin", bufs=2))
        ypool = ctx.enter_context(tc.tile_pool(name="yin", bufs=2))
        hpool = ctx.enter_context(tc.tile_pool(name="hb", bufs=1))
        ynpool = ctx.enter_context(tc.tile_pool(name="yn", bufs=1))
        qpool = ctx.enter_context(tc.tile_pool(name="qb", bufs=1))
        kpool = ctx.enter_context(tc.tile_pool(name="kb", bufs=1))
        vpool = ctx.enter_context(tc.tile_pool(name="vb", bufs=1))
        epool = ctx.enter_context(tc.tile_pool(name="eb", bufs=1))
        opool = ctx.enter_context(tc.tile_pool(name="ob", bufs=2))
        rzpool = ctx.enter_context(tc.tile_pool(name="rz", bufs=2))
        outpool = ctx.enter_context(tc.tile_pool(name="outb", bufs=3))
        spool = ctx.enter_context(tc.tile_pool(name="small", bufs=2))
        pmm = ctx.enter_context(tc.tile_pool(name="pmm", bufs=3, space="PSUM"))
        pzb = ctx.enter_context(tc.tile_pool(name="pzb", bufs=1, space="PSUM"))
        pgs = ctx.enter_context(tc.tile_pool(name="pgs", bufs=1, space="PSUM"))

        def load_one(dst_sb, src, eng=None):
            v = src.rearrange("p (t n) -> p t n", n=HW)
            for t in range(CT):
                (eng or nc.sync).dma_start(dst_sb[:, t, :], v[:, t, :])

        def gn_stats_pre(src_sb, stats, half, uid):
            """DVE per-partition statistics into stats[:, half]: [mean, var,
            mean^2] per channel tile (mean^2 is filled later, combined)."""
            for t in range(CT):
                bns = spool.tile([P, 2, 6], F32, tag=f"bns{uid}")
                for h2 in range(2):
                    nc.vector.bn_stats(
                        bns[:, h2, :], src_sb[:, t, h2 * 512 : (h2 + 1) * 512]
                    )
                nc.vector.bn_aggr(stats[:, half, t, 0:2], bns[:])

        def gn_stats_post(stats, uid, halves=(0, 2)):
            """Cross-partition group aggregation (one tiny PE matmul per half
            range) and the affine constants a, mb — all DVE, no ACT. halves
            selects a [lo, lo+n) slice of the x/y dim so the batch-0 x post
            can run before y statistics are even finished."""
            lo, n = halves[0], halves[1] - halves[0]
            sl = slice(lo, lo + n)
            nc.vector.tensor_tensor(
                stats[:, sl, :, 2], stats[:, sl, :, 0], stats[:, sl, :, 0],
                op=ALU.mult,
            )
            gps = pgs.tile([P, 2, CT, 3], F32, tag="gs")
            nc.tensor.matmul(gps[:, sl], amat_sb[:], stats[:, sl],
                             start=True, stop=True)
            g = spool.tile([P, 2, CT, 3], F32, tag=f"g{uid}")
            nc.vector.tensor_copy(g[:, sl], gps[:, sl])
            # var_g = E[var] + E[mean^2] - E[mean]^2  (equal-count partitions)
            msq = spool.tile([P, 2, CT], F32, tag=f"msq{uid}")
            nc.vector.tensor_tensor(msq[:, sl], g[:, sl, :, 0], g[:, sl, :, 0],
                                    op=ALU.mult)
            var = spool.tile([P, 2, CT], F32, tag=f"var{uid}")
            nc.vector.tensor_tensor(var[:, sl], g[:, sl, :, 1], g[:, sl, :, 2],
                                    op=ALU.add)
            nc.vector.tensor_tensor(var[:, sl], var[:, sl], msq[:, sl],
                                    op=ALU.subtract)
            nc.vector.tensor_scalar(var[:, sl], var[:, sl], EPS, None, op0=ALU.add)
            # rstd = rsqrt(var+eps): bit-trick seed + one Newton step gives
            # ~0.2% relative error, far below the fp8 noise floor
            seed = spool.tile([P, 2, CT], I32, tag=f"sd{uid}")
            nc.vector.tensor_scalar(
                seed[:, sl], var[:, sl].bitcast(I32), 1, None,
                op0=ALU.arith_shift_right
            )
            nc.vector.tensor_scalar(
                seed[:, sl], seed[:, sl], 0xFFFFFFFF, None, op0=ALU.bitwise_xor
            )
            nc.vector.tensor_scalar(
                seed[:, sl], seed[:, sl], MAGIC + 1, None, op0=ALU.add
            )
            r0 = seed[:, sl].bitcast(F32)
            t4 = spool.tile([P, 2, CT], F32, tag=f"t4{uid}")
            nc.vector.tensor_tensor(t4[:, sl], r0, r0, op=ALU.mult)
            nc.vector.tensor_tensor(t4[:, sl], t4[:, sl], var[:, sl], op=ALU.mult)
            nc.vector.tensor_scalar(t4[:, sl], t4[:, sl], -0.5, 1.5,
                                    op0=ALU.mult, op1=ALU.add)
            rstd = spool.tile([P, 2, CT], F32, tag=f"rs{uid}")
            nc.vector.tensor_tensor(rstd[:, sl], r0, t4[:, sl], op=ALU.mult)
            # a = rstd*gamma ; mb = beta - mean*a   (rows: scales 0:2, biases 2:4)
            a = spool.tile([P, 2, CT], F32, tag=f"a{uid}")
            nc.vector.tensor_tensor(
                a[:, sl], rstd[:, sl],
                prm_sb[:, R_SCALE + lo : R_SCALE + lo + n, :], op=ALU.mult
            )
            mb = spool.tile([P, 2, CT], F32, tag=f"mb{uid}")
            nc.vector.tensor_tensor(mb[:, sl], g[:, sl, :, 0], a[:, sl], op=ALU.mult)
            nc.vector.tensor_tensor(
                mb[:, sl], prm_sb[:, R_BIAS + lo : R_BIAS + lo + n, :], mb[:, sl],
                op=ALU.subtract
            )
            return a, mb

        def gn_apply(src_sb, dst_sb, st, half, engines=None):
            a, mb = st
            engs = engines or [nc.gpsimd] * CT
            for t in range(CT):
                if engs[t] is nc.scalar:
                    nc.scalar.activation(
                        dst_sb[:, t, :], src_sb[:, t, :], AF.Identity,
                        bias=mb[:, half, t : t + 1], scale=a[:, half, t : t + 1],
                    )
                else:
                    engs[t].tensor_scalar(
                        dst_sb[:, t, :], src_sb[:, t, :],
                        a[:, half, t : t + 1], mb[:, half, t : t + 1],
                        op0=ALU.mult, op1=ALU.add,
                    )

        # ---- prologue: x first (gates everything), then wk, y, rest ----
        x_sb = xpool.tile([P, CT, HW], BF16, tag="x")
        xv0 = xs[0].rearrange("p (t n) -> p t n", n=HW)
        for t in range(CT):
            (nc.sync if t < 2 else nc.gpsimd).dma_start(x_sb[:, t, :], xv0[:, t, :])
        prm_sb = cpool.tile([P, 6, CT], F32)
        nc.sync.dma_start(prm_sb[:], prm.rearrange("p (q t) -> p q t", t=CT))
        amat_sb = cpool.tile([P, P], F32)
        nc.sync.dma_start(amat_sb[:], amat[:])
        w_sb = {}

        def loadw(name, ap):
            t = wpool.tile([P, CT, C], F8, tag=name)
            nc.gpsimd.dma_start(t[:], ap.rearrange("p (t o) -> p t o", o=C))
            w_sb[name] = t

        loadw("wk", wk)
        y_sb = ypool.tile([P, CT, HW], BF16, tag="y")
        load_one(y_sb, ys[0], eng=nc.scalar)
        ones_mat = cpool.tile([P, 2, P], F8)
        nc.sync.dma_start(ones_mat[:], aps["ones"].rearrange("p (s q) -> p s q", s=2))
        ebias = cpool.tile([P, 1], F32)
        nc.vector.memset(ebias[:], EXP_BIAS)
        loadw("wv", wv)
        loadw("wq", wq)
        loadw("wp", wp)

        stats0 = spool.tile([P, 2, CT, 3], F32, tag="st0")
        gn_stats_pre(x_sb, stats0, 0, uid="x0")
        st_x0 = gn_stats_post(stats0, uid="b0x", halves=(0, 1))
        gn_stats_pre(y_sb, stats0, 1, uid="y0")
        nxt = {}

        for b in range(BPC):
            outv = out[b].rearrange("p (t n) -> p t n", n=HW)

            xcur, ycur = x_sb, y_sb
            if nxt:
                h_sb, yn_sb = nxt.pop("h"), nxt.pop("yn")
            else:
                # batch 0 is latency-critical: apply h on DVE+ACT in parallel;
                # the y-side post/apply is deferred past the k/vT matmuls so
                # its tiny PE matmul never blocks them in the in-order queue
                h_sb = hpool.tile([P, CT, HW], F8, tag="h")
                gn_apply(xcur, h_sb, st_x0, 0,
                         engines=[nc.vector, nc.scalar, nc.vector, nc.scalar])
                yn_sb = None

            # ---- k = Wk h  (k[c_out, i]); one paired evac per mt on DVE ----
            k_sb = kpool.tile([P, CT, HW], F8, tag="k")
            for mt in range(CT):
                ps = pmm.tile([P, 2 * IBS], F32, tag="ps")
                for nh in range(IB):
                    for kp in range(0, CT, 2):
                        nc.tensor.matmul(
                            ps[:, nh * IBS : (nh + 1) * IBS],
                            w_sb["wk"][:, kp : kp + 2, mt * P : (mt + 1) * P],
                            h_sb[:, kp : kp + 2, nh * IBS : (nh + 1) * IBS],
                            start=(kp == 0), stop=(kp == CT - 2),
                            perf_mode=DR,
                        )
                nc.scalar.copy(k_sb[:, mt, :], ps[:])

            # ---- vT[j, c_out] = h^T WvT; paired evac per jt-pair on ACT ----
            vT_sb = vpool.tile([P, JT, C], F8, tag="vT")
            for jp in range(0, JT, 2):
                ps = pmm.tile([P, 2 * C], F32, tag="ps")
                for u in range(2):
                    for kp in range(0, CT, 2):
                        nc.tensor.matmul(
                            ps[:, u * C : (u + 1) * C],
                            h_sb[:, kp : kp + 2, (jp + u) * P : (jp + u + 1) * P],
                            w_sb["wv"][:, kp : kp + 2, :],
                            start=(kp == 0), stop=(kp == CT - 2),
                            perf_mode=DR,
                        )
                nc.vector.tensor_copy(vT_sb[:, jp : jp + 2, :], ps[:])

            if yn_sb is None:
                # batch-0 y GroupNorm: post + apply now that k/vT are queued
                st_y0 = gn_stats_post(stats0, uid="b0y", halves=(1, 2))
                yn_sb = ynpool.tile([P, CT, HW], F8, tag="yn")
                gn_apply(ycur, yn_sb, st_y0, 1,
                         engines=[nc.gpsimd, nc.vector, nc.gpsimd, nc.vector])

            # ---- q = Wq y_ + bq; paired evac per mt on ACT (bias add) ----
            q_sb = qpool.tile([P, CT, HW], F8, tag="q")
            for mt in range(CT):
                ps = pmm.tile([P, 2 * IBS], F32, tag="ps")
                for nh in range(IB):
                    for kp in range(0, CT, 2):
                        nc.tensor.matmul(
                            ps[:, nh * IBS : (nh + 1) * IBS],
                            w_sb["wq"][:, kp : kp + 2, mt * P : (mt + 1) * P],
                            yn_sb[:, kp : kp + 2, nh * IBS : (nh + 1) * IBS],
                            start=(kp == 0), stop=(kp == CT - 2),
                            perf_mode=DR,
                        )
                nc.scalar.activation(
                    q_sb[:, mt, :], ps[:], AF.Identity,
                    bias=prm_sb[:, R_BQ, mt : mt + 1],
                )

            # prefetch next batch + its DVE-only stats; the tiny stats
            # matmul is deferred so it never blocks this batch's attention
            if b + 1 < BPC:
                xn_t = xpool.tile([P, CT, HW], BF16, tag="x")
                load_one(xn_t, xs[b + 1])
                yn_t = ypool.tile([P, CT, HW], BF16, tag="y")
                load_one(yn_t, ys[b + 1], eng=nc.gpsimd)
                statsn = spool.tile([P, 2, CT, 3], F32, tag="st1")
                gn_stats_pre(xn_t, statsn, 0, uid=f"x{b+1}")
                gn_stats_pre(yn_t, statsn, 1, uid=f"y{b+1}")

            # ---- attention; r-projection pipelined one i-block behind ----
            rdefer = []

            def emit_r(ib, mps=(0, 2)):
                isl2 = slice(ib * IBS, (ib + 1) * IBS)
                o0_p = rdefer[0]
                for mp in mps:
                    ps = pmm.tile([P, 2 * IBS], F32, tag="ps")
                    for u in range(2):
                        for cp in range(0, CT, 2):
                            nc.tensor.matmul(
                                ps[:, u * IBS : (u + 1) * IBS],
                                w_sb["wp"][:, cp : cp + 2, (mp + u) * P : (mp + u + 1) * P],
                                o0_p[:, cp : cp + 2, :],
                                start=(cp == 0), stop=(cp == CT - 2),
                                perf_mode=DR,
                            )
                    for u in range(2):
                        ot = outpool.tile([P, IBS], BF16, tag="ot")
                        nc.vector.scalar_tensor_tensor(
                            ot[:], ps[:, u * IBS : (u + 1) * IBS],
                            prm_sb[:, R_BPP, mp + u : mp + u + 1],
                            xcur[:, mp + u, isl2], op0=ALU.add, op1=ALU.add,
                        )
                        nc.sync.dma_start(outv[:, mp + u, isl2], ot[:])

            for ib in range(IB):
                isl = slice(ib * IBS, (ib + 1) * IBS)
                e_sb = epool.tile([P, JT, IBS], F8, tag="e")
                zb = pzb.tile([P, IBS], F32, tag="zb")
                # S and exp per key-tile pair; Z (ones-matmul column sums of
                # E) lags one pair behind so the in-order PE never waits on
                # the ACT-engine exp.
                for jp in range(0, JT, 2):
                    ps = pmm.tile([P, 2 * IBS], F32, tag="ps")
                    for u in range(2):
                        for kp in range(0, CT, 2):
                            nc.tensor.matmul(
                                ps[:, u * IBS : (u + 1) * IBS],
                                k_sb[:, kp : kp + 2, (jp + u) * P : (jp + u + 1) * P],
                                q_sb[:, kp : kp + 2, isl],
                                start=(kp == 0), stop=(kp == CT - 2),
                                perf_mode=DR,
                            )
                    # E = exp(S / sqrt(c) - 3); logits are O(1), no max needed
                    nc.scalar.activation(e_sb[:, jp : jp + 2, :], ps[:], AF.Exp,
                                         bias=ebias[:], scale=SM_SCALE)
                    if jp >= 4:
                        jz = jp - 4
                        nc.tensor.matmul(
                            zb[:], ones_mat[:], e_sb[:, jz : jz + 2, :],
                            start=(jz == 0), stop=False,
                            perf_mode=DR,
                        )
                # r-projection of the previous i-block fills the PE gap
                # while the last exp pairs drain on ACT
                if ib > 0:
                    emit_r(ib - 1)
                    rdefer.pop(0)
                for jz in (JT - 4, JT - 2):
                    nc.tensor.matmul(
                        zb[:], ones_mat[:], e_sb[:, jz : jz + 2, :],
                        start=False, stop=(jz == JT - 2),
                        perf_mode=DR,
                    )
                rzb = rzpool.tile([P, IBS], F32, tag="rzb")
                nc.vector.reciprocal_approx_fast(rzb[:], zb[:])

                o0_sb = opool.tile([P, CT, IBS], F8, tag="o0")
                for cp in range(0, CT, 2):
                    ps = pmm.tile([P, 2 * IBS], F32, tag="ps")
                    for u in range(2):
                        for jp in range(0, JT, 2):
                            nc.tensor.matmul(
                                ps[:, u * IBS : (u + 1) * IBS],
                                vT_sb[:, jp : jp + 2, (cp + u) * P : (cp + u + 1) * P],
                                e_sb[:, jp : jp + 2, :],
                                start=(jp == 0), stop=(jp == JT - 2),
                                perf_mode=DR,
                            )
                    for u in range(2):
                        nc.vector.tensor_tensor(
                            o0_sb[:, cp + u, :], ps[:, u * IBS : (u + 1) * IBS],
                            rzb[:], op=ALU.mult,
                        )
                rdefer.append(o0_sb)
                # next batch's GroupNorm chain launches after the first
                # i-block so GpSimd has the whole second i-block to finish
                # h/yn before the next batch's projections need them
                if ib == 0 and b + 1 < BPC:
                    stn = gn_stats_post(statsn, uid=f"b{b+1}")
                    h_n = hpool.tile([P, CT, HW], F8, tag="h")
                    gn_apply(xn_t, h_n, stn, 0)
                    yn_n = ynpool.tile([P, CT, HW], F8, tag="yn")
                    gn_apply(yn_t, yn_n, stn, 1)
                    nxt = {"h": h_n, "yn": yn_n}
            emit_r(IB - 1)
            if b + 1 < BPC:
                x_sb, y_sb = xn_t, yn_t


_CACHE = {}


def _build():
    if "nc" in _CACHE:
        return _CACHE["nc"]
    nc = bacc.Bacc("TRN2", target_bir_lowering=False, debug=False)
    aps = {
        "xs": nc.dram_tensor("xs", [BPC, P, CT * HW], BF16, kind="ExternalInput").ap(),
        "ys": nc.dram_tensor("ys", [BPC, P, CT * HW], BF16, kind="ExternalInput").ap(),
        "wqT": nc.dram_tensor("wqT", [P, CT * C], F8, kind="ExternalInput").ap(),
        "wkT": nc.dram_tensor("wkT", [P, CT * C], F8, kind="ExternalInput").ap(),
        "wvT": nc.dram_tensor("wvT", [P, CT * C], F8, kind="ExternalInput").ap(),
        "wpT": nc.dram_tensor("wpT", [P, CT * C], F8, kind="ExternalInput").ap(),
        "prm": nc.dram_tensor("prm", [P, 6 * CT], F32, kind="ExternalInput").ap(),
        "amat": nc.dram_tensor("amat", [P, P], F32, kind="ExternalInput").ap(),
        "ones": nc.dram_tensor("ones", [P, 2 * P], F8, kind="ExternalInput").ap(),
        "out": nc.dram_tensor("out", [BPC, P, CT * HW], BF16, kind="ExternalOutput").ap(),
    }
    with tile.TileContext(nc) as tc:
        _emit(tc, aps)
    nc.compile()
    _CACHE["nc"] = nc
    return nc


def _pack_chw(a, dtype):
    """[*, C, HW] -> [*, P, CT*HW] matching SBUF layout c = t*128 + p."""
    lead = a.shape[:-2]
    a = a.reshape(*lead, CT, P, HW)
    a = np.moveaxis(a, -3, -2)          # [..., P, CT, HW]
    return np.ascontiguousarray(
        a.reshape(*lead, P, CT * HW).astype(dtype)
    )


def _unpack_chw(a):
    """[*, P, CT*HW] -> [*, C, HW]."""
    a = np.asarray(a).astype(np.float32)
    lead = a.shape[:-2]
    a = a.reshape(*lead, P, CT, HW)
    a = np.moveaxis(a, -2, -3)          # [..., CT, P, HW]
    return np.ascontiguousarray(a.reshape(*lead, CT * P, HW))


def _host_inputs(x, y, norm_scale, norm_bias, norm1_scale, norm1_bias,
                 wq, bq, wk, bk, wv, bv, wp, bp):
    f = lambda a: np.ascontiguousarray(np.asarray(a, dtype=np.float32))
    x = f(x).reshape(B, C, HW)
    y = f(y).reshape(B, C, HW)
    wq, wk, wv, wp = f(wq), f(wk), f(wv), f(wp)
    # bk cancels in softmax; bv folds into bp' because softmax rows sum to 1
    bpp = f(bp) + wp @ f(bv)
    # rows: [gn_scale, gn1_scale, gn_bias, gn1_bias, bq, bp'] so the x/y
    # scale (and bias) pairs are adjacent for combined-stats processing
    prm = np.stack([f(norm_scale), f(norm1_scale), f(norm_bias), f(norm1_bias),
                    f(bq), bpp]).astype(np.float32)
    # [6, C] -> [P, 6*CT] matching prm_sb[p, q, t]
    prm = np.ascontiguousarray(
        prm.reshape(6, CT, P).transpose(2, 0, 1).reshape(P, 6 * CT)
    )
    amat = np.zeros((P, P), np.float32)
    for g in range(P // GSIZE):
        amat[g * GSIZE : (g + 1) * GSIZE, g * GSIZE : (g + 1) * GSIZE] = 1.0 / GSIZE

    def packw(w):
        # wT [c_in, c_out] -> [P, CT*C] matching w_sb[p, kt, o]; e4m3 with
        # clip to the TRN +-240 max (values beyond round to inf)
        wT8 = np.clip(w.T, -240.0, 240.0).astype(ml_dtypes.float8_e4m3)
        return np.ascontiguousarray(
            wT8.reshape(CT, P, C).transpose(1, 0, 2).reshape(P, CT * C)
        )

    shared = {
        "wqT": packw(wq), "wkT": packw(wk), "wvT": packw(wv), "wpT": packw(wp),
        "prm": prm, "amat": amat,
        "ones": np.ones((P, 2 * P), ml_dtypes.float8_e4m3),
    }
    in_maps = []
    for core in range(NCORES):
        sl = slice(core * BPC, (core + 1) * BPC)
        in_maps.append({
            "xs": _pack_chw(x[sl], ml_dtypes.bfloat16),
            "ys": _pack_chw(y[sl], ml_dtypes.bfloat16),
            **shared,
        })
    return in_maps


def _run(in_maps, trace=False):
    nc = _build()
    res = run_bass_kernel_spmd(
        nc, in_maps, core_ids=list(range(NCORES)), trace=trace
    )
    out = np.concatenate(
        [_unpack_chw(res.results[i]["out"]) for i in range(NCORES)], axis=0
    ).reshape(B, C, H, W)
    return out, res


def kernel(**inputs):
    in_maps = _host_inputs(**inputs)
    out, _ = _run(in_maps, trace=False)
    return out


# revision 28
# speedup vs baseline: 1.1942x; 1.1942x over previous
"""AttnBlock fusion kernel for Trainium2 (Bass/Tile), 8 NeuronCores.

Reference computation (per batch element b; c=512 channels, hw=1024 spatial):
    h  = GroupNorm(32, c)(x) ; k = Wk h + bk ; v = Wv h + bv
    y_ = GroupNorm(32, c)(y) ; q = Wq y_ + bq
    attn = softmax_j(q^T k / sqrt(c)) ; o = v @ attn^T ; out = x + Wp o + bp

Sharding: pure data parallel over batch (16 batches / 8 cores = 2 each).

Key algebraic moves (exactness notes inline):
  * bk cancels in softmax over j (adds a per-i constant to logits) -> dropped.
  * bv contributes bv * sum_j(attn) = bv exactly -> folded into
    bp' = bp + Wp @ bv on the host.
  * v is produced directly transposed (vT[j,c]) by the projection matmul,
    and attention scores are computed as S[j,i]; no on-chip transposes.
  * softmax denominator: ones matmul gives the partition-broadcast
    column sum of exp(S) directly in PSUM.

Precision strategy (tolerance is rel_l2 < 2e-2; this lands ~7e-3):
  * x/y stream in as bf16 (halves input DMA); GroupNorm statistics and all
    PSUM accumulation stay fp32.
  * All matmuls run in fp8 e4m3 with perf_mode=DoubleRow (2 fp8 weights
    per PE cell -> 2 contraction rows/instruction). Operands are laid out
    [P, ksub, free] so a pair of 128-row k-subtiles feeds each instruction.
  * exp uses bias=-3 so e^(logit-3) stays far below the e4m3 +-240
    limit (TRN fp8e4 overflows to inf, which would poison the matmul).
    The -3 cancels exactly in softmax normalization.
  * Output is written bf16 and upcast on host.

Engine budget notes: DVE/ACT instructions cost ~0.3us fixed overhead each,
so PSUM results are paired into [P, 2, 512] tiles (two banks) and
evacuated with a single op. GroupNorm statistics post-processing runs
entirely on DVE (bit-trick rsqrt) so the ACT table never leaves Exp
during attention; GroupNorm application runs on GpSimd (SBUF-only).
"""

import math
import os
import sys

import numpy as np

for _p in ("/opt/trn_rl_repo", "/root/.axon_site/_ro/trn_rl_repo"):
    if os.path.isdir(_p) and _p not in sys.path:
        sys.path.append(_p)

import ml_dtypes

import concourse.bass as bass
import concourse.bacc as bacc
import concourse.mybir as mybir
import concourse.tile as tile
from concourse.bass_utils import run_bass_kernel_spmd

F32 = mybir.dt.float32
I32 = mybir.dt.int32
BF16 = mybir.dt.bfloat16
F8 = mybir.dt.float8e4
AF = mybir.ActivationFunctionType
ALU = mybir.AluOpType
DR = mybir.MatmulPerfMode.DoubleRow

B, C, H, W = 16, 512, 32, 32
HW = H * W                  # 1024
NCORES = 8
BPC = B // NCORES           # 2 batches per core
P = 128                     # SBUF partitions
CT = C // P                 # 4 channel tiles
JT = HW // P                # 8 key-position tiles
IBS = 512                   # i-block size (query positions per block)
IB = HW // IBS              # 2 i-blocks
GROUPS = 32
GSIZE = C // GROUPS         # 16 channels per group
EPS = 1e-6
SM_SCALE = float(int(C) ** -0.5)
EXP_BIAS = -3.0             # keeps exp() outputs well inside e4m3 range
MAGIC = 0x5F3759DF          # fp32 rsqrt seed

# prm rows: gn scales (x,y), gn biases (x,y), bq, bp'
R_SCALE, R_BIAS, R_BQ, R_BPP = 0, 2, 4, 5


def _emit(tc, aps):
    nc = tc.nc
    xs, ys, wq, wk, wv, wp, prm, amat, out = (
        aps["xs"], aps["ys"], aps["wqT"], aps["wkT"], aps["wvT"], aps["wpT"],
        aps["prm"], aps["amat"], aps["out"],
    )

    from contextlib import ExitStack

    with ExitStack() as ctx:
        cpool = ctx.enter_context(tc.tile_pool(name="const", bufs=1))
        wpool = ctx.enter_context(tc.tile_pool(name="w", bufs=1))
        xpool = ctx.enter_context(tc.tile_pool(name="xin", bufs=2))
        ypool = ctx.enter_context(tc.tile_pool(name="yin", bufs=2))
        hpool = ctx.enter_context(tc.tile_pool(name="hb", bufs=1))
        ynpool = ctx.enter_context(tc.tile_pool(name="yn", bufs=1))
        qpool = ctx.enter_context(tc.tile_pool(name="qb", bufs=1))
        kpool = ctx.enter_context(tc.tile_pool(name="kb", bufs=1))
        vpool = ctx.enter_context(tc.tile_pool(name="vb", bufs=1))
        epool = ctx.enter_context(tc.tile_pool(name="eb", bufs=1))
        opool = ctx.enter_context(tc.tile_pool(name="ob", bufs=2))
        rzpool = ctx.enter_context(tc.tile_pool(name="rz", bufs=2))
        outpool = ctx.enter_context(tc.tile_pool(name="outb", bufs=3))
        spool = ctx.enter_context(tc.tile_pool(name="small", bufs=2))
        pmm = ctx.enter_context(tc.tile_pool(name="pmm", bufs=3, space="PSUM"))
        pzb = ctx.enter_context(tc.tile_pool(name="pzb", bufs=1, space="PSUM"))
        pgs = ctx.enter_context(tc.tile_pool(name="pgs", bufs=1, space="PSUM"))

        def load_one(dst_sb, src, eng=None):
            v = src.rearrange("p (t n) -> p t n", n=HW)
            for t in range(CT):
                (eng or nc.sync).dma_start(dst_sb[:, t, :], v[:, t, :])

        def gn_stats_pre(src_sb, stats, half, uid):
            """DVE per-partition statistics into stats[:, half]: [mean, var,
            mean^2] per channel tile (mean^2 is filled later, combined)."""
            for t in range(CT):
                bns = spool.tile([P, 2, 6], F32, tag=f"bns{uid}")
                for h2 in range(2):
                    nc.vector.bn_stats(
                        bns[:, h2, :], src_sb[:, t, h2 * 512 : (h2 + 1) * 512]
                    )
                nc.vector.bn_aggr(stats[:, half, t, 0:2], bns[:])

        def gn_stats_post(stats, uid, halves=(0, 2)):
            """Cross-partition group aggregation (one tiny PE matmul per half
            range) and the affine constants a, mb — all DVE, no ACT. halves
            selects a [lo, lo+n) slice of the x/y dim so the batch-0 x post
            can run before y statistics are even finished."""
            lo, n = halves[0], halves[1] - halves[0]
            sl = slice(lo, lo + n)
            nc.vector.tensor_tensor(
                stats[:, sl, :, 2], stats[:, sl, :, 0], stats[:, sl, :, 0],
                op=ALU.mult,
            )
            gps = pgs.tile([P, 2, CT, 3], F32, tag="gs")
            nc.tensor.matmul(gps[:, sl], amat_sb[:], stats[:, sl],
                             start=True, stop=True)
            g = spool.tile([P, 2, CT, 3], F32, tag=f"g{uid}")
            nc.vector.tensor_copy(g[:, sl], gps[:, sl])
            # var_g = E[var] + E[mean^2] - E[mean]^2  (equal-count partitions)
            msq = spool.tile([P, 2, CT], F32, tag=f"msq{uid}")
            nc.vector.tensor_tensor(msq[:, sl], g[:, sl, :, 0], g[:, sl, :, 0],
                                    op=ALU.mult)
            var = spool.tile([P, 2, CT], F32, tag=f"var{uid}")
            nc.vector.tensor_tensor(var[:, sl], g[:, sl, :, 1], g[:, sl, :, 2],
                                    op=ALU.add)
            nc.vector.tensor_tensor(var[:, sl], var[:, sl], msq[:, sl],
                                    op=ALU.subtract)
            nc.vector.tensor_scalar(var[:, sl], var[:, sl], EPS, None, op0=ALU.add)
            # rstd = rsqrt(var+eps): bit-trick seed + one Newton step gives
            # ~0.2% relative error, far below the fp8 noise floor
            seed = spool.tile([P, 2, CT], I32, tag=f"sd{uid}")
            nc.vector.tensor_scalar(
                seed[:, sl], var[:, sl].bitcast(I32), 1, None,
                op0=ALU.arith_shift_right
            )
            nc.vector.tensor_scalar(
                seed[:, sl], seed[:, sl], 0xFFFFFFFF, None, op0=ALU.bitwise_xor
            )
            nc.vector.tensor_scalar(
                seed[:, sl], seed[:, sl], MAGIC + 1, None, op0=ALU.add
            )
            r0 = seed[:, sl].bitcast(F32)
            t4 = spool.tile([P, 2, CT], F32, tag=f"t4{uid}")
            nc.vector.tensor_tensor(t4[:, sl], r0, r0, op=ALU.mult)
            nc.vector.tensor_tensor(t4[:, sl], t4[:, sl], var[:, sl], op=ALU.mult)
            nc.vector.tensor_scalar(t4[:, sl], t4[:, sl], -0.5, 1.5,
                                    op0=ALU.mult, op1=ALU.add)
            rstd = spool.tile([P, 2, CT], F32, tag=f"rs{uid}")
            nc.vector.tensor_tensor(rstd[:, sl], r0, t4[:, sl], op=ALU.mult)
            # a = rstd*gamma ; mb = beta - mean*a   (rows: scales 0:2, biases 2:4)
            a = spool.tile([P, 2, CT], F32, tag=f"a{uid}")
            nc.vector.tensor_tensor(
                a[:, sl], rstd[:, sl],
                prm_sb[:, R_SCALE + lo : R_SCALE + lo + n, :], op=ALU.mult
            )
            mb = spool.tile([P, 2, CT], F32, tag=f"mb{uid}")
            nc.vector.tensor_tensor(mb[:, sl], g[:, sl, :, 0], a[:, sl], op=ALU.mult)
            nc.vector.tensor_tensor(
                mb[:, sl], prm_sb[:, R_BIAS + lo : R_BIAS + lo + n, :], mb[:, sl],
                op=ALU.subtract
            )
            return a, mb

        def gn_apply(src_sb, dst_sb, st, half, engines=None):
            a, mb = st
            engs = engines or [nc.gpsimd] * CT
            for t in range(CT):
                if engs[t] is nc.scalar:
                    nc.scalar.activation(
                        dst_sb[:, t, :], src_sb[:, t, :], AF.Identity,
                        bias=mb[:, half, t : t + 1], scale=a[:, half, t : t + 1],
                    )
                else:
                    engs[t].tensor_scalar(
                        dst_sb[:, t, :], src_sb[:, t, :],
                        a[:, half, t : t + 1], mb[:, half, t : t + 1],
                        op0=ALU.mult, op1=ALU.add,
                    )

        # ---- prologue: x first (gates everything), then wk, y, rest ----
        x_sb = xpool.tile([P, CT, HW], BF16, tag="x")
        xv0 = xs[0].rearrange("p (t n) -> p t n", n=HW)
        for t in range(CT):
            (nc.sync if t < 2 else nc.gpsimd).dma_start(x_sb[:, t, :], xv0[:, t, :])
        prm_sb = cpool.tile([P, 6, CT], F32)
        nc.sync.dma_start(prm_sb[:], prm.rearrange("p (q t) -> p q t", t=CT))
        amat_sb = cpool.tile([P, P], F32)
        nc.sync.dma_start(amat_sb[:], amat[:])
        w_sb = {}

        def loadw(name, ap):
            t = wpool.tile([P, CT, C], F8, tag=name)
            nc.gpsimd.dma_start(t[:], ap.rearrange("p (t o) -> p t o", o=C))
            w_sb[name] = t

        loadw("wk", wk)
        y_sb = ypool.tile([P, CT, HW], BF16, tag="y")
        load_one(y_sb, ys[0], eng=nc.scalar)
        ones_mat = cpool.tile([P, 2, P], F8)
        nc.sync.dma_start(ones_mat[:], aps["ones"].rearrange("p (s q) -> p s q", s=2))
        ebias = cpool.tile([P, 1], F32)
        nc.vector.memset(ebias[:], EXP_BIAS)
        loadw("wv", wv)
        loadw("wq", wq)
        loadw("wp", wp)

        stats0 = spool.tile([P, 2, CT, 3], F32, tag="st0")
        gn_stats_pre(x_sb, stats0, 0, uid="x0")
        st_x0 = gn_stats_post(stats0, uid="b0x", halves=(0, 1))
        gn_stats_pre(y_sb, stats0, 1, uid="y0")
        nxt = {}

        for b in range(BPC):
            outv = out[b].rearrange("p (t n) -> p t n", n=HW)

            xcur, ycur = x_sb, y_sb
            if nxt:
                h_sb, yn_sb = nxt.pop("h"), nxt.pop("yn")
            else:
                # batch 0 is latency-critical: apply h on DVE+ACT in parallel;
                # the y-side post/apply is deferred past the k/vT matmuls so
                # its tiny PE matmul never blocks them in the in-order queue
                h_sb = hpool.tile([P, CT, HW], F8, tag="h")
                gn_apply(xcur, h_sb, st_x0, 0,
                         engines=[nc.vector, nc.scalar, nc.vector, nc.scalar])
                yn_sb = None

            # ---- k = Wk h  (k[c_out, i]); one paired evac per mt on DVE ----
            k_sb = kpool.tile([P, CT, HW], F8, tag="k")
            for mt in range(CT):
                ps = pmm.tile([P, 2 * IBS], F32, tag="ps")
                for nh in range(IB):
                    for kp in range(0, CT, 2):
                        nc.tensor.matmul(
                            ps[:, nh * IBS : (nh + 1) * IBS],
                            w_sb["wk"][:, kp : kp + 2, mt * P : (mt + 1) * P],
                            h_sb[:, kp : kp + 2, nh * IBS : (nh + 1) * IBS],
                            start=(kp == 0), stop=(kp == CT - 2),
                            perf_mode=DR,
                        )
                nc.scalar.copy(k_sb[:, mt, :], ps[:])

            # ---- vT[j, c_out] = h^T WvT; paired evac per jt-pair on ACT ----
            vT_sb = vpool.tile([P, JT, C], F8, tag="vT")
            for jp in range(0, JT, 2):
                ps = pmm.tile([P, 2 * C], F32, tag="ps")
                for u in range(2):
                    for kp in range(0, CT, 2):
                        nc.tensor.matmul(
                            ps[:, u * C : (u + 1) * C],
                            h_sb[:, kp : kp + 2, (jp + u) * P : (jp + u + 1) * P],
                            w_sb["wv"][:, kp : kp + 2, :],
                            start=(kp == 0), stop=(kp == CT - 2),
                            perf_mode=DR,
                        )
                nc.scalar.copy(vT_sb[:, jp : jp + 2, :], ps[:])

            if yn_sb is None:
                # batch-0 y GroupNorm: post + apply now that k/vT are queued
                st_y0 = gn_stats_post(stats0, uid="b0y", halves=(1, 2))
                yn_sb = ynpool.tile([P, CT, HW], F8, tag="yn")
                gn_apply(ycur, yn_sb, st_y0, 1,
                         engines=[nc.scalar, nc.gpsimd, nc.scalar, nc.gpsimd])

            # ---- q = Wq y_ + bq; paired evac per mt on ACT (bias add) ----
            q_sb = qpool.tile([P, CT, HW], F8, tag="q")
            for mt in range(CT):
                ps = pmm.tile([P, 2 * IBS], F32, tag="ps")
                for nh in range(IB):
                    for kp in range(0, CT, 2):
                        nc.tensor.matmul(
                            ps[:, nh * IBS : (nh + 1) * IBS],
                            w_sb["wq"][:, kp : kp + 2, mt * P : (mt + 1) * P],
                            yn_sb[:, kp : kp + 2, nh * IBS : (nh + 1) * IBS],
                            start=(kp == 0), stop=(kp == CT - 2),
                            perf_mode=DR,
                        )
                nc.scalar.activation(
                    q_sb[:, mt, :], ps[:], AF.Identity,
                    bias=prm_sb[:, R_BQ, mt : mt + 1],
                )

            # prefetch next batch + its DVE-only stats; the tiny stats
            # matmul is deferred so it never blocks this batch's attention
            if b + 1 < BPC:
                xn_t = xpool.tile([P, CT, HW], BF16, tag="x")
                load_one(xn_t, xs[b + 1])
                yn_t = ypool.tile([P, CT, HW], BF16, tag="y")
                load_one(yn_t, ys[b + 1], eng=nc.gpsimd)
                statsn = spool.tile([P, 2, CT, 3], F32, tag="st1")
                gn_stats_pre(xn_t, statsn, 0, uid=f"x{b+1}")
                gn_stats_pre(yn_t, statsn, 1, uid=f"y{b+1}")

            # ---- attention; r-projection pipelined one i-block behind ----
            rdefer = []

            def emit_r(ib, mps=(0, 2)):
                isl2 = slice(ib * IBS, (ib + 1) * IBS)
                o0_p = rdefer[0]
                for mp in mps:
                    ps = pmm.tile([P, 2 * IBS], F32, tag="ps")
                    for u in range(2):
                        for cp in range(0, CT, 2):
                            nc.tensor.matmul(
                                ps[:, u * IBS : (u + 1) * IBS],
                                w_sb["wp"][:, cp : cp + 2, (mp + u) * P : (mp + u + 1) * P],
                                o0_p[:, cp : cp + 2, :],
                                start=(cp == 0), stop=(cp == CT - 2),
                                perf_mode=DR,
                            )
                    for u in range(2):
                        ot = outpool.tile([P, IBS], BF16, tag="ot")
                        nc.vector.scalar_tensor_tensor(
                            ot[:], ps[:, u * IBS : (u + 1) * IBS],
                            prm_sb[:, R_BPP, mp + u : mp + u + 1],
                            xcur[:, mp + u, isl2], op0=ALU.add, op1=ALU.add,
                        )
                        nc.sync.dma_start(outv[:, mp + u, isl2], ot[:])

            for ib in range(IB):
                isl = slice(ib * IBS, (ib + 1) * IBS)
                e_sb = epool.tile([P, JT, IBS], F8, tag="e")
                zb = pzb.tile([P, IBS], F32, tag="zb")
                # S and exp per key-tile pair; Z (ones-matmul column sums of
                # E) lags one pair behind so the in-order PE never waits on
                # the ACT-engine exp.
                for jp in range(0, JT, 2):
                    ps = pmm.tile([P, 2 * IBS], F32, tag="ps")
                    for u in range(2):
                        for kp in range(0, CT, 2):
                            nc.tensor.matmul(
                                ps[:, u * IBS : (u + 1) * IBS],
                                k_sb[:, kp : kp + 2, (jp + u) * P : (jp + u + 1) * P],
                                q_sb[:, kp : kp + 2, isl],
                                start=(kp == 0), stop=(kp == CT - 2),
                                perf_mode=DR,
                            )
                    # E = exp(S / sqrt(c) - 3); logits are O(1), no max needed
                    nc.scalar.activation(e_sb[:, jp : jp + 2, :], ps[:], AF.Exp,
                                         bias=ebias[:], scale=SM_SCALE)
                    if jp >= 4:
                        jz = jp - 4
                        nc.tensor.matmul(
                            zb[:], ones_mat[:], e_sb[:, jz : jz + 2, :],
                            start=(jz == 0), stop=False,
                            perf_mode=DR,
                        )
                # r-projection of the previous i-block fills the PE gap
                # while the last exp pairs drain on ACT
                if ib > 0:
                    emit_r(ib - 1)
                    rdefer.pop(0)
                for jz in (JT - 4, JT - 2):
                    nc.tensor.matmul(
                        zb[:], ones_mat[:], e_sb[:, jz : jz + 2, :],
                        start=False, stop=(jz == JT - 2),
                        perf_mode=DR,
                    )
                rzb = rzpool.tile([P, IBS], F32, tag="rzb")
                nc.vector.reciprocal_approx_fast(rzb[:], zb[:])

                o0_sb = opool.tile([P, CT, IBS], F8, tag="o0")
                for cp in range(0, CT, 2):
                    ps = pmm.tile([P, 2 * IBS], F32, tag="ps")
                    for u in range(2):
                        for jp in range(0, JT, 2):
                            nc.tensor.matmul(
                                ps[:, u * IBS : (u + 1) * IBS],
                                vT_sb[:, jp : jp + 2, (cp + u) * P : (cp + u + 1) * P],
                                e_sb[:, jp : jp + 2, :],
                                start=(jp == 0), stop=(jp == JT - 2),
                                perf_mode=DR,
                            )
                    for u in range(2):
                        nc.vector.tensor_tensor(
                            o0_sb[:, cp + u, :], ps[:, u * IBS : (u + 1) * IBS],
                            rzb[:], op=ALU.mult,
                        )
                rdefer.append(o0_sb)
                # next batch's GroupNorm chain launches after the first
                # i-block so GpSimd has the whole second i-block to finish
                # h/yn before the next batch's projections need them
                if ib == 0 and b + 1 < BPC:
                    stn = gn_stats_post(statsn, uid=f"b{b+1}")
                    h_n = hpool.tile([P, CT, HW], F8, tag="h")
                    gn_apply(xn_t, h_n, stn, 0)
                    yn_n = ynpool.tile([P, CT, HW], F8, tag="yn")
                    gn_apply(yn_t, yn_n, stn, 1)
                    nxt = {"h": h_n, "yn": yn_n}
            emit_r(IB - 1)
            if b + 1 < BPC:
                x_sb, y_sb = xn_t, yn_t


_CACHE = {}


def _build():
    if "nc" in _CACHE:
        return _CACHE["nc"]
    nc = bacc.Bacc("TRN2", target_bir_lowering=False, debug=False)
    aps = {
        "xs": nc.dram_tensor("xs", [BPC, P, CT * HW], BF16, kind="ExternalInput").ap(),
        "ys": nc.dram_tensor("ys", [BPC, P, CT * HW], BF16, kind="ExternalInput").ap(),
        "wqT": nc.dram_tensor("wqT", [P, CT * C], F8, kind="ExternalInput").ap(),
        "wkT": nc.dram_tensor("wkT", [P, CT * C], F8, kind="ExternalInput").ap(),
        "wvT": nc.dram_tensor("wvT", [P, CT * C], F8, kind="ExternalInput").ap(),
        "wpT": nc.dram_tensor("wpT", [P, CT * C], F8, kind="ExternalInput").ap(),
        "prm": nc.dram_tensor("prm", [P, 6 * CT], F32, kind="ExternalInput").ap(),
        "amat": nc.dram_tensor("amat", [P, P], F32, kind="ExternalInput").ap(),
        "ones": nc.dram_tensor("ones", [P, 2 * P], F8, kind="ExternalInput").ap(),
        "out": nc.dram_tensor("out", [BPC, P, CT * HW], BF16, kind="ExternalOutput").ap(),
    }
    with tile.TileContext(nc) as tc:
        _emit(tc, aps)
    nc.compile()
    _CACHE["nc"] = nc
    return nc


def _pack_chw(a, dtype):
    """[*, C, HW] -> [*, P, CT*HW] matching SBUF layout c = t*128 + p."""
    lead = a.shape[:-2]
    a = a.reshape(*lead, CT, P, HW)
    a = np.moveaxis(a, -3, -2)          # [..., P, CT, HW]
    return np.ascontiguousarray(
        a.reshape(*lead, P, CT * HW).astype(dtype)
    )


def _unpack_chw(a):
    """[*, P, CT*HW] -> [*, C, HW]."""
    a = np.asarray(a).astype(np.float32)
    lead = a.shape[:-2]
    a = a.reshape(*lead, P, CT, HW)
    a = np.moveaxis(a, -2, -3)          # [..., CT, P, HW]
    return np.ascontiguousarray(a.reshape(*lead, CT * P, HW))


def _host_inputs(x, y, norm_scale, norm_bias, norm1_scale, norm1_bias,
                 wq, bq, wk, bk, wv, bv, wp, bp):
    f = lambda a: np.ascontiguousarray(np.asarray(a, dtype=np.float32))
    x = f(x).reshape(B, C, HW)
    y = f(y).reshape(B, C, HW)
    wq, wk, wv, wp = f(wq), f(wk), f(wv), f(wp)
    # bk cancels in softmax; bv folds into bp' because softmax rows sum to 1
    bpp = f(bp) + wp @ f(bv)
    # rows: [gn_scale, gn1_scale, gn_bias, gn1_bias, bq, bp'] so the x/y
    # scale (and bias) pairs are adjacent for combined-stats processing
    prm = np.stack([f(norm_scale), f(norm1_scale), f(norm_bias), f(norm1_bias),
                    f(bq), bpp]).astype(np.float32)
    # [6, C] -> [P, 6*CT] matching prm_sb[p, q, t]
    prm = np.ascontiguousarray(
        prm.reshape(6, CT, P).transpose(2, 0, 1).reshape(P, 6 * CT)
    )
    amat = np.zeros((P, P), np.float32)
    for g in range(P // GSIZE):
        amat[g * GSIZE : (g + 1) * GSIZE, g * GSIZE : (g + 1) * GSIZE] = 1.0 / GSIZE

    def packw(w):
        # wT [c_in, c_out] -> [P, CT*C] matching w_sb[p, kt, o]; e4m3 with
        # clip to the TRN +-240 max (values beyond round to inf)
        wT8 = np.clip(w.T, -240.0, 240.0).astype(ml_dtypes.float8_e4m3)
        return np.ascontiguousarray(
            wT8.reshape(CT, P, C).transpose(1, 0, 2).reshape(P, CT * C)
        )

    shared = {
        "wqT": packw(wq), "wkT": packw(wk), "wvT": packw(wv), "wpT": packw(wp),
        "prm": prm, "amat": amat,
        "ones": np.ones((P, 2 * P), ml_dtypes.float8_e4m3),
    }
    in_maps = []
    for core in range(NCORES):
        sl = slice(core * BPC, (core + 1) * BPC)
        in_maps.append({
            "xs": _pack_chw(x[sl], ml_dtypes.bfloat16),
            "ys": _pack_chw(y[sl], ml_dtypes.bfloat16),
            **shared,
        })
    return in_maps


def _run(in_maps, trace=False):
    nc = _build()
    res = run_bass_kernel_spmd(
        nc, in_maps, core_ids=list(range(NCORES)), trace=trace
    )
    out = np.concatenate(
        [_unpack_chw(res.results[i]["out"]) for i in range(NCORES)], axis=0
    ).reshape(B, C, H, W)
    return out, res


def kernel(**inputs):
    in_maps = _host_inputs(**inputs)
    out, _ = _run(in_maps, trace=False)
    return out


# revision 33
# speedup vs baseline: 1.2004x; 1.0052x over previous
"""AttnBlock fusion kernel for Trainium2 (Bass/Tile), 8 NeuronCores.

Reference computation (per batch element b; c=512 channels, hw=1024 spatial):
    h  = GroupNorm(32, c)(x) ; k = Wk h + bk ; v = Wv h + bv
    y_ = GroupNorm(32, c)(y) ; q = Wq y_ + bq
    attn = softmax_j(q^T k / sqrt(c)) ; o = v @ attn^T ; out = x + Wp o + bp

Sharding: pure data parallel over batch (16 batches / 8 cores = 2 each).

Key algebraic moves (exactness notes inline):
  * bk cancels in softmax over j (adds a per-i constant to logits) -> dropped.
  * bv contributes bv * sum_j(attn) = bv exactly -> folded into
    bp' = bp + Wp @ bv on the host.
  * v is produced directly transposed (vT[j,c]) by the projection matmul,
    and attention scores are computed as S[j,i]; no on-chip transposes.
  * softmax denominator: ones matmul gives the partition-broadcast
    column sum of exp(S) directly in PSUM.

Precision strategy (tolerance is rel_l2 < 2e-2; this lands ~7e-3):
  * x/y stream in as bf16 (halves input DMA); GroupNorm statistics and all
    PSUM accumulation stay fp32.
  * All matmuls run in fp8 e4m3 with perf_mode=DoubleRow (2 fp8 weights
    per PE cell -> 2 contraction rows/instruction). Operands are laid out
    [P, ksub, free] so a pair of 128-row k-subtiles feeds each instruction.
  * exp uses bias=-3 so e^(logit-3) stays far below the e4m3 +-240
    limit (TRN fp8e4 overflows to inf, which would poison the matmul).
    The -3 cancels exactly in softmax normalization.
  * Output is written bf16 and upcast on host.

Engine budget notes: DVE/ACT instructions cost ~0.3us fixed overhead each,
so PSUM results are paired into [P, 2, 512] tiles (two banks) and
evacuated with a single op. GroupNorm statistics post-processing runs
entirely on DVE (bit-trick rsqrt) so the ACT table never leaves Exp
during attention; GroupNorm application runs on GpSimd (SBUF-only).
"""

import math
import os
import sys

import numpy as np

for _p in ("/opt/trn_rl_repo", "/root/.axon_site/_ro/trn_rl_repo"):
    if os.path.isdir(_p) and _p not in sys.path:
        sys.path.append(_p)

import ml_dtypes

import concourse.bass as bass
import concourse.bacc as bacc
import concourse.mybir as mybir
import concourse.tile as tile
from concourse.bass_utils import run_bass_kernel_spmd

F32 = mybir.dt.float32
I32 = mybir.dt.int32
BF16 = mybir.dt.bfloat16
F8 = mybir.dt.float8e4
AF = mybir.ActivationFunctionType
ALU = mybir.AluOpType
DR = mybir.MatmulPerfMode.DoubleRow

B, C, H, W = 16, 512, 32, 32
HW = H * W                  # 1024
NCORES = 8
BPC = B // NCORES           # 2 batches per core
P = 128                     # SBUF partitions
CT = C // P                 # 4 channel tiles
JT = HW // P                # 8 key-position tiles
IBS = 512                   # i-block size (query positions per block)
IB = HW // IBS              # 2 i-blocks
GROUPS = 32
GSIZE = C // GROUPS         # 16 channels per group
EPS = 1e-6
SM_SCALE = float(int(C) ** -0.5)
EXP_BIAS = -3.0             # keeps exp() outputs well inside e4m3 range
MAGIC = 0x5F3759DF          # fp32 rsqrt seed

# prm rows: gn scales (x,y), gn biases (x,y), bq, bp'
R_SCALE, R_BIAS, R_BQ, R_BPP = 0, 2, 4, 5


def _emit(tc, aps):
    nc = tc.nc
    xs, ys, wq, wk, wv, wp, prm, amat, out = (
        aps["xs"], aps["ys"], aps["wqT"], aps["wkT"], aps["wvT"], aps["wpT"],
        aps["prm"], aps["amat"], aps["out"],
    )

    from contextlib import ExitStack

    with ExitStack() as ctx:
        cpool = ctx.enter_context(tc.tile_pool(name="const", bufs=1))
        wpool = ctx.enter_context(tc.tile_pool(name="w", bufs=1))
        xpool = ctx.enter_context(tc.tile_pool(name="xin", bufs=2))
        ypool = ctx.enter_context(tc.tile_pool(name="yin", bufs=2))
        hpool = ctx.enter_context(tc.tile_pool(name="hb", bufs=1))
        ynpool = ctx.enter_context(tc.tile_pool(name="yn", bufs=1))
        qpool = ctx.enter_context(tc.tile_pool(name="qb", bufs=1))
        kpool = ctx.enter_context(tc.tile_pool(name="kb", bufs=1))
        vpool = ctx.enter_context(tc.tile_pool(name="vb", bufs=1))
        epool = ctx.enter_context(tc.tile_pool(name="eb", bufs=1))
        opool = ctx.enter_context(tc.tile_pool(name="ob", bufs=2))
        rzpool = ctx.enter_context(tc.tile_pool(name="rz", bufs=2))
        outpool = ctx.enter_context(tc.tile_pool(name="outb", bufs=3))
        spool = ctx.enter_context(tc.tile_pool(name="small", bufs=2))
        pmm = ctx.enter_context(tc.tile_pool(name="pmm", bufs=3, space="PSUM"))
        pzb = ctx.enter_context(tc.tile_pool(name="pzb", bufs=1, space="PSUM"))
        pgs = ctx.enter_context(tc.tile_pool(name="pgs", bufs=1, space="PSUM"))

        def load_one(dst_sb, src, eng=None):
            v = src.rearrange("p (t n) -> p t n", n=HW)
            for t in range(CT):
                (eng or nc.sync).dma_start(dst_sb[:, t, :], v[:, t, :])

        def gn_stats_pre(src_sb, stats, half, uid):
            """DVE per-partition statistics into stats[:, half]: [mean, var,
            mean^2] per channel tile (mean^2 is filled later, combined)."""
            for t in range(CT):
                bns = spool.tile([P, 2, 6], F32, tag=f"bns{uid}")
                for h2 in range(2):
                    nc.vector.bn_stats(
                        bns[:, h2, :], src_sb[:, t, h2 * 512 : (h2 + 1) * 512]
                    )
                nc.vector.bn_aggr(stats[:, half, t, 0:2], bns[:])

        def gn_stats_act(src_sb, stats, half, uid):
            """GroupNorm partial stats on the ACT engine (accum_out sums):
            fills stats[:, half] with [mean, E[x^2], 0], which feeds the same
            group-combine formula with the mean^2 slot zeroed."""
            tr = spool.tile([P, HW], F8, tag="ytr")
            sums = spool.tile([P, CT, 2], F32, tag=f"ys{uid}")
            for t in range(CT):
                nc.scalar.activation(tr[:], src_sb[:, t, :], AF.Identity,
                                     accum_out=sums[:, t, 0:1])
            for t in range(CT):
                nc.scalar.activation(tr[:], src_sb[:, t, :], AF.Square,
                                     accum_out=sums[:, t, 1:2])
            nc.vector.tensor_scalar(stats[:, half, :, 0], sums[:, :, 0],
                                    1.0 / HW, None, op0=ALU.mult)
            nc.vector.tensor_scalar(stats[:, half, :, 1], sums[:, :, 1],
                                    1.0 / HW, None, op0=ALU.mult)
            nc.vector.memset(stats[:, half, :, 2], 0.0)

        def gn_stats_post(stats, uid, halves=(0, 2), premixed=False):
            """Cross-partition group aggregation (one tiny PE matmul per half
            range) and the affine constants a, mb — all DVE, no ACT. halves
            selects a [lo, lo+n) slice of the x/y dim so the batch-0 x post
            can run before y statistics are even finished."""
            lo, n = halves[0], halves[1] - halves[0]
            sl = slice(lo, lo + n)
            if not premixed:
                nc.vector.tensor_tensor(
                    stats[:, sl, :, 2], stats[:, sl, :, 0], stats[:, sl, :, 0],
                    op=ALU.mult,
                )
            gps = pgs.tile([P, 2, CT, 3], F32, tag="gs")
            nc.tensor.matmul(gps[:, sl], amat_sb[:], stats[:, sl],
                             start=True, stop=True)
            g = spool.tile([P, 2, CT, 3], F32, tag=f"g{uid}")
            nc.vector.tensor_copy(g[:, sl], gps[:, sl])
            # var_g = E[var] + E[mean^2] - E[mean]^2  (equal-count partitions)
            msq = spool.tile([P, 2, CT], F32, tag=f"msq{uid}")
            nc.vector.tensor_tensor(msq[:, sl], g[:, sl, :, 0], g[:, sl, :, 0],
                                    op=ALU.mult)
            var = spool.tile([P, 2, CT], F32, tag=f"var{uid}")
            nc.vector.tensor_tensor(var[:, sl], g[:, sl, :, 1], g[:, sl, :, 2],
                                    op=ALU.add)
            nc.vector.tensor_tensor(var[:, sl], var[:, sl], msq[:, sl],
                                    op=ALU.subtract)
            nc.vector.tensor_scalar(var[:, sl], var[:, sl], EPS, None, op0=ALU.add)
            # rstd = rsqrt(var+eps): bit-trick seed + one Newton step gives
            # ~0.2% relative error, far below the fp8 noise floor
            seed = spool.tile([P, 2, CT], I32, tag=f"sd{uid}")
            nc.vector.tensor_scalar(
                seed[:, sl], var[:, sl].bitcast(I32), 1, None,
                op0=ALU.arith_shift_right
            )
            nc.vector.tensor_scalar(
                seed[:, sl], seed[:, sl], 0xFFFFFFFF, None, op0=ALU.bitwise_xor
            )
            nc.vector.tensor_scalar(
                seed[:, sl], seed[:, sl], MAGIC + 1, None, op0=ALU.add
            )
            r0 = seed[:, sl].bitcast(F32)
            t4 = spool.tile([P, 2, CT], F32, tag=f"t4{uid}")
            nc.vector.tensor_tensor(t4[:, sl], r0, r0, op=ALU.mult)
            nc.vector.tensor_tensor(t4[:, sl], t4[:, sl], var[:, sl], op=ALU.mult)
            nc.vector.tensor_scalar(t4[:, sl], t4[:, sl], -0.5, 1.5,
                                    op0=ALU.mult, op1=ALU.add)
            rstd = spool.tile([P, 2, CT], F32, tag=f"rs{uid}")
            nc.vector.tensor_tensor(rstd[:, sl], r0, t4[:, sl], op=ALU.mult)
            # a = rstd*gamma ; mb = beta - mean*a   (rows: scales 0:2, biases 2:4)
            a = spool.tile([P, 2, CT], F32, tag=f"a{uid}")
            nc.vector.tensor_tensor(
                a[:, sl], rstd[:, sl],
                prm_sb[:, R_SCALE + lo : R_SCALE + lo + n, :], op=ALU.mult
            )
            mb = spool.tile([P, 2, CT], F32, tag=f"mb{uid}")
            nc.vector.tensor_tensor(mb[:, sl], g[:, sl, :, 0], a[:, sl], op=ALU.mult)
            nc.vector.tensor_tensor(
                mb[:, sl], prm_sb[:, R_BIAS + lo : R_BIAS + lo + n, :], mb[:, sl],
                op=ALU.subtract
            )
            return a, mb

        def gn_apply(src_sb, dst_sb, st, half, engines=None):
            a, mb = st
            engs = engines or [nc.gpsimd] * CT
            for t in range(CT):
                if engs[t] is nc.scalar:
                    nc.scalar.activation(
                        dst_sb[:, t, :], src_sb[:, t, :], AF.Identity,
                        bias=mb[:, half, t : t + 1], scale=a[:, half, t : t + 1],
                    )
                else:
                    engs[t].tensor_scalar(
                        dst_sb[:, t, :], src_sb[:, t, :],
                        a[:, half, t : t + 1], mb[:, half, t : t + 1],
                        op0=ALU.mult, op1=ALU.add,
                    )

        # ---- prologue: x first (gates everything), then wk, y, rest ----
        x_sb = xpool.tile([P, CT, HW], BF16, tag="x")
        xv0 = xs[0].rearrange("p (t n) -> p t n", n=HW)
        for t in range(CT):
            (nc.sync if t < 2 else nc.scalar).dma_start(x_sb[:, t, :], xv0[:, t, :])
        prm_sb = cpool.tile([P, 6, CT], F32)
        nc.sync.dma_start(prm_sb[:], prm.rearrange("p (q t) -> p q t", t=CT))
        amat_sb = cpool.tile([P, P], F32)
        nc.sync.dma_start(amat_sb[:], amat[:])
        w_sb = {}

        def loadw(name, ap):
            t = wpool.tile([P, CT, C], F8, tag=name)
            nc.gpsimd.dma_start(t[:], ap.rearrange("p (t o) -> p t o", o=C))
            w_sb[name] = t

        loadw("wk", wk)
        y_sb = ypool.tile([P, CT, HW], BF16, tag="y")
        load_one(y_sb, ys[0], eng=nc.gpsimd)
        ones_mat = cpool.tile([P, 2, P], F8)
        nc.sync.dma_start(ones_mat[:], aps["ones"].rearrange("p (s q) -> p s q", s=2))
        ebias = cpool.tile([P, 1], F32)
        nc.vector.memset(ebias[:], EXP_BIAS)
        loadw("wv", wv)
        loadw("wq", wq)
        loadw("wp", wp)

        stats0 = spool.tile([P, 2, CT, 3], F32, tag="st0")
        gn_stats_pre(x_sb, stats0, 0, uid="x0")
        st_x0 = gn_stats_post(stats0, uid="b0x", halves=(0, 1))
        gn_stats_act(y_sb, stats0, 1, uid="y0")
        nxt = {}

        for b in range(BPC):
            outv = out[b].rearrange("p (t n) -> p t n", n=HW)

            xcur, ycur = x_sb, y_sb
            if nxt:
                h_sb, yn_sb = nxt.pop("h"), nxt.pop("yn")
            else:
                # batch 0 is latency-critical: apply h on DVE+ACT in parallel;
                # the y-side post/apply is deferred past the k/vT matmuls so
                # its tiny PE matmul never blocks them in the in-order queue
                h_sb = hpool.tile([P, CT, HW], F8, tag="h")
                gn_apply(xcur, h_sb, st_x0, 0,
                         engines=[nc.vector, nc.gpsimd, nc.vector, nc.gpsimd])
                yn_sb = None

            # ---- k = Wk h  (k[c_out, i]); one paired evac per mt on DVE ----
            k_sb = kpool.tile([P, CT, HW], F8, tag="k")
            for mt in range(CT):
                ps = pmm.tile([P, 2 * IBS], F32, tag="ps")
                for nh in range(IB):
                    for kp in range(0, CT, 2):
                        nc.tensor.matmul(
                            ps[:, nh * IBS : (nh + 1) * IBS],
                            w_sb["wk"][:, kp : kp + 2, mt * P : (mt + 1) * P],
                            h_sb[:, kp : kp + 2, nh * IBS : (nh + 1) * IBS],
                            start=(kp == 0), stop=(kp == CT - 2),
                            perf_mode=DR,
                        )
                nc.scalar.copy(k_sb[:, mt, :], ps[:])

            # ---- vT[j, c_out] = h^T WvT; paired evac per jt-pair on ACT ----
            vT_sb = vpool.tile([P, JT, C], F8, tag="vT")
            for jp in range(0, JT, 2):
                ps = pmm.tile([P, 2 * C], F32, tag="ps")
                for u in range(2):
                    for kp in range(0, CT, 2):
                        nc.tensor.matmul(
                            ps[:, u * C : (u + 1) * C],
                            h_sb[:, kp : kp + 2, (jp + u) * P : (jp + u + 1) * P],
                            w_sb["wv"][:, kp : kp + 2, :],
                            start=(kp == 0), stop=(kp == CT - 2),
                            perf_mode=DR,
                        )
                nc.scalar.copy(vT_sb[:, jp : jp + 2, :], ps[:])

            if yn_sb is None:
                # batch-0 y GroupNorm: post + apply now that k/vT are queued
                st_y0 = gn_stats_post(stats0, uid="b0y", halves=(1, 2),
                                      premixed=True)
                yn_sb = ynpool.tile([P, CT, HW], F8, tag="yn")
                gn_apply(ycur, yn_sb, st_y0, 1,
                         engines=[nc.gpsimd, nc.vector, nc.gpsimd, nc.vector])

            # ---- q = Wq y_ + bq; paired evac per mt on ACT (bias add) ----
            q_sb = qpool.tile([P, CT, HW], F8, tag="q")
            for mt in range(CT):
                ps = pmm.tile([P, 2 * IBS], F32, tag="ps")
                for nh in range(IB):
                    for kp in range(0, CT, 2):
                        nc.tensor.matmul(
                            ps[:, nh * IBS : (nh + 1) * IBS],
                            w_sb["wq"][:, kp : kp + 2, mt * P : (mt + 1) * P],
                            yn_sb[:, kp : kp + 2, nh * IBS : (nh + 1) * IBS],
                            start=(kp == 0), stop=(kp == CT - 2),
                            perf_mode=DR,
                        )
                nc.scalar.activation(
                    q_sb[:, mt, :], ps[:], AF.Identity,
                    bias=prm_sb[:, R_BQ, mt : mt + 1],
                )

            # prefetch next batch + its DVE-only stats; the tiny stats
            # matmul is deferred so it never blocks this batch's attention
            if b + 1 < BPC:
                xn_t = xpool.tile([P, CT, HW], BF16, tag="x")
                load_one(xn_t, xs[b + 1])
                yn_t = ypool.tile([P, CT, HW], BF16, tag="y")
                load_one(yn_t, ys[b + 1], eng=nc.gpsimd)
                statsn = spool.tile([P, 2, CT, 3], F32, tag="st1")
                gn_stats_pre(xn_t, statsn, 0, uid=f"x{b+1}")
                gn_stats_pre(yn_t, statsn, 1, uid=f"y{b+1}")

            # ---- attention; r-projection pipelined one i-block behind ----
            rdefer = []

            def emit_r(ib, mps=(0, 2)):
                isl2 = slice(ib * IBS, (ib + 1) * IBS)
                o0_p = rdefer[0]
                for mp in mps:
                    ps = pmm.tile([P, 2 * IBS], F32, tag="ps")
                    for u in range(2):
                        for cp in range(0, CT, 2):
                            nc.tensor.matmul(
                                ps[:, u * IBS : (u + 1) * IBS],
                                w_sb["wp"][:, cp : cp + 2, (mp + u) * P : (mp + u + 1) * P],
                                o0_p[:, cp : cp + 2, :],
                                start=(cp == 0), stop=(cp == CT - 2),
                                perf_mode=DR,
                            )
                    for u in range(2):
                        ot = outpool.tile([P, IBS], BF16, tag="ot")
                        nc.vector.scalar_tensor_tensor(
                            ot[:], ps[:, u * IBS : (u + 1) * IBS],
                            prm_sb[:, R_BPP, mp + u : mp + u + 1],
                            xcur[:, mp + u, isl2], op0=ALU.add, op1=ALU.add,
                        )
                        nc.sync.dma_start(outv[:, mp + u, isl2], ot[:])

            for ib in range(IB):
                isl = slice(ib * IBS, (ib + 1) * IBS)
                e_sb = epool.tile([P, JT, IBS], F8, tag="e")
                zb = pzb.tile([P, IBS], F32, tag="zb")
                # S and exp per key-tile pair; Z (ones-matmul column sums of
                # E) lags one pair behind so the in-order PE never waits on
                # the ACT-engine exp.
                for jp in range(0, JT, 2):
                    ps = pmm.tile([P, 2 * IBS], F32, tag="ps")
                    for u in range(2):
                        for kp in range(0, CT, 2):
                            nc.tensor.matmul(
                                ps[:, u * IBS : (u + 1) * IBS],
                                k_sb[:, kp : kp + 2, (jp + u) * P : (jp + u + 1) * P],
                                q_sb[:, kp : kp + 2, isl],
                                start=(kp == 0), stop=(kp == CT - 2),
                                perf_mode=DR,
                            )
                    # E = exp(S / sqrt(c) - 3); logits are O(1), no max needed
                    nc.scalar.activation(e_sb[:, jp : jp + 2, :], ps[:], AF.Exp,
                                         bias=ebias[:], scale=SM_SCALE)
                    if jp >= 4:
                        jz = jp - 4
                        nc.tensor.matmul(
                            zb[:], ones_mat[:], e_sb[:, jz : jz + 2, :],
                            start=(jz == 0), stop=False,
                            perf_mode=DR,
                        )
                # r-projection of the previous i-block fills the PE gap
                # while the last exp pairs drain on ACT
                if ib > 0:
                    emit_r(ib - 1)
                    rdefer.pop(0)
                for jz in (JT - 4, JT - 2):
                    nc.tensor.matmul(
                        zb[:], ones_mat[:], e_sb[:, jz : jz + 2, :],
                        start=False, stop=(jz == JT - 2),
                        perf_mode=DR,
                    )
                rzb = rzpool.tile([P, IBS], F32, tag="rzb")
                nc.vector.reciprocal_approx_fast(rzb[:], zb[:])

                o0_sb = opool.tile([P, CT, IBS], F8, tag="o0")
                for cp in range(0, CT, 2):
                    ps = pmm.tile([P, 2 * IBS], F32, tag="ps")
                    for u in range(2):
                        for jp in range(0, JT, 2):
                            nc.tensor.matmul(
                                ps[:, u * IBS : (u + 1) * IBS],
                                vT_sb[:, jp : jp + 2, (cp + u) * P : (cp + u + 1) * P],
                                e_sb[:, jp : jp + 2, :],
                                start=(jp == 0), stop=(jp == JT - 2),
                                perf_mode=DR,
                            )
                    for u in range(2):
                        nc.vector.tensor_tensor(
                            o0_sb[:, cp + u, :], ps[:, u * IBS : (u + 1) * IBS],
                            rzb[:], op=ALU.mult,
                        )
                rdefer.append(o0_sb)
                # next batch's GroupNorm chain launches after the first
                # i-block so GpSimd has the whole second i-block to finish
                # h/yn before the next batch's projections need them
                if ib == 0 and b + 1 < BPC:
                    stn = gn_stats_post(statsn, uid=f"b{b+1}")
                    h_n = hpool.tile([P, CT, HW], F8, tag="h")
                    gn_apply(xn_t, h_n, stn, 0)
                    yn_n = ynpool.tile([P, CT, HW], F8, tag="yn")
                    gn_apply(yn_t, yn_n, stn, 1)
                    nxt = {"h": h_n, "yn": yn_n}
            emit_r(IB - 1)
            if b + 1 < BPC:
                x_sb, y_sb = xn_t, yn_t


_CACHE = {}


def _build():
    if "nc" in _CACHE:
        return _CACHE["nc"]
    nc = bacc.Bacc("TRN2", target_bir_lowering=False, debug=False)
    aps = {
        "xs": nc.dram_tensor("xs", [BPC, P, CT * HW], BF16, kind="ExternalInput").ap(),
        "ys": nc.dram_tensor("ys", [BPC, P, CT * HW], BF16, kind="ExternalInput").ap(),
        "wqT": nc.dram_tensor("wqT", [P, CT * C], F8, kind="ExternalInput").ap(),
        "wkT": nc.dram_tensor("wkT", [P, CT * C], F8, kind="ExternalInput").ap(),
        "wvT": nc.dram_tensor("wvT", [P, CT * C], F8, kind="ExternalInput").ap(),
        "wpT": nc.dram_tensor("wpT", [P, CT * C], F8, kind="ExternalInput").ap(),
        "prm": nc.dram_tensor("prm", [P, 6 * CT], F32, kind="ExternalInput").ap(),
        "amat": nc.dram_tensor("amat", [P, P], F32, kind="ExternalInput").ap(),
        "ones": nc.dram_tensor("ones", [P, 2 * P], F8, kind="ExternalInput").ap(),
        "out": nc.dram_tensor("out", [BPC, P, CT * HW], BF16, kind="ExternalOutput").ap(),
    }
    with tile.TileContext(nc) as tc:
        _emit(tc, aps)
    nc.compile()
    _CACHE["nc"] = nc
    return nc


def _pack_chw(a, dtype):
    """[*, C, HW] -> [*, P, CT*HW] matching SBUF layout c = t*128 + p."""
    lead = a.shape[:-2]
    a = a.reshape(*lead, CT, P, HW)
    a = np.moveaxis(a, -3, -2)          # [..., P, CT, HW]
    return np.ascontiguousarray(
        a.reshape(*lead, P, CT * HW).astype(dtype)
    )


def _unpack_chw(a):
    """[*, P, CT*HW] -> [*, C, HW]."""
    a = np.asarray(a).astype(np.float32)
    lead = a.shape[:-2]
    a = a.reshape(*lead, P, CT, HW)
    a = np.moveaxis(a, -2, -3)          # [..., CT, P, HW]
    return np.ascontiguousarray(a.reshape(*lead, CT * P, HW))


def _host_inputs(x, y, norm_scale, norm_bias, norm1_scale, norm1_bias,
                 wq, bq, wk, bk, wv, bv, wp, bp):
    f = lambda a: np.ascontiguousarray(np.asarray(a, dtype=np.float32))
    x = f(x).reshape(B, C, HW)
    y = f(y).reshape(B, C, HW)
    wq, wk, wv, wp = f(wq), f(wk), f(wv), f(wp)
    # bk cancels in softmax; bv folds into bp' because softmax rows sum to 1
    bpp = f(bp) + wp @ f(bv)
    # rows: [gn_scale, gn1_scale, gn_bias, gn1_bias, bq, bp'] so the x/y
    # scale (and bias) pairs are adjacent for combined-stats processing
    prm = np.stack([f(norm_scale), f(norm1_scale), f(norm_bias), f(norm1_bias),
                    f(bq), bpp]).astype(np.float32)
    # [6, C] -> [P, 6*CT] matching prm_sb[p, q, t]
    prm = np.ascontiguousarray(
        prm.reshape(6, CT, P).transpose(2, 0, 1).reshape(P, 6 * CT)
    )
    amat = np.zeros((P, P), np.float32)
    for g in range(P // GSIZE):
        amat[g * GSIZE : (g + 1) * GSIZE, g * GSIZE : (g + 1) * GSIZE] = 1.0 / GSIZE

    def packw(w):
        # wT [c_in, c_out] -> [P, CT*C] matching w_sb[p, kt, o]; e4m3 with
        # clip to the TRN +-240 max (values beyond round to inf)
        wT8 = np.clip(w.T, -240.0, 240.0).astype(ml_dtypes.float8_e4m3)
        return np.ascontiguousarray(
            wT8.reshape(CT, P, C).transpose(1, 0, 2).reshape(P, CT * C)
        )

    shared = {
        "wqT": packw(wq), "wkT": packw(wk), "wvT": packw(wv), "wpT": packw(wp),
        "prm": prm, "amat": amat,
        "ones": np.ones((P, 2 * P), ml_dtypes.float8_e4m3),
    }
    in_maps = []
    for core in range(NCORES):
        sl = slice(core * BPC, (core + 1) * BPC)
        in_maps.append({
            "xs": _pack_chw(x[sl], ml_dtypes.bfloat16),
            "ys": _pack_chw(y[sl], ml_dtypes.bfloat16),
            **shared,
        })
    return in_maps


def _run(in_maps, trace=False):
    nc = _build()
    res = run_bass_kernel_spmd(
        nc, in_maps, core_ids=list(range(NCORES)), trace=trace
    )
    out = np.concatenate(
        [_unpack_chw(res.results[i]["out"]) for i in range(NCORES)], axis=0
    ).reshape(B, C, H, W)
    return out, res


def kernel(**inputs):
    in_maps = _host_inputs(**inputs)
    out, _ = _run(in_maps, trace=False)
    return out


# revision 42
# speedup vs baseline: 1.2409x; 1.0337x over previous
"""AttnBlock fusion kernel for Trainium2 (Bass/Tile), 8 NeuronCores.

Reference computation (per batch element b; c=512 channels, hw=1024 spatial):
    h  = GroupNorm(32, c)(x) ; k = Wk h + bk ; v = Wv h + bv
    y_ = GroupNorm(32, c)(y) ; q = Wq y_ + bq
    attn = softmax_j(q^T k / sqrt(c)) ; o = v @ attn^T ; out = x + Wp o + bp

Sharding: pure data parallel over batch (16 batches / 8 cores = 2 each).

Key algebraic moves (exactness notes inline):
  * bk cancels in softmax over j (adds a per-i constant to logits) -> dropped.
  * bv contributes bv * sum_j(attn) = bv exactly -> folded into
    bp' = bp + Wp @ bv on the host.
  * v is produced directly transposed (vT[j,c]) by the projection matmul,
    and attention scores are computed as S[j,i]; no on-chip transposes.
  * softmax denominator: ones matmul gives the partition-broadcast
    column sum of exp(S) directly in PSUM.

Precision strategy (tolerance is rel_l2 < 2e-2; this lands ~7e-3):
  * x/y stream in as bf16 (halves input DMA); GroupNorm statistics and all
    PSUM accumulation stay fp32.
  * All matmuls run in fp8 e4m3 with perf_mode=DoubleRow (2 fp8 weights
    per PE cell -> 2 contraction rows/instruction). Operands are laid out
    [P, ksub, free] so a pair of 128-row k-subtiles feeds each instruction.
  * exp uses bias=-3 so e^(logit-3) stays far below the e4m3 +-240
    limit (TRN fp8e4 overflows to inf, which would poison the matmul).
    The -3 cancels exactly in softmax normalization.
  * Output is written bf16 and upcast on host.

Engine budget notes: DVE/ACT instructions cost ~0.3us fixed overhead each,
so PSUM results are paired into [P, 2, 512] tiles (two banks) and
evacuated with a single op. GroupNorm statistics post-processing runs
entirely on DVE (bit-trick rsqrt) so the ACT table never leaves Exp
during attention; GroupNorm application runs on GpSimd (SBUF-only).
"""

import math
import os
import sys

import numpy as np

for _p in ("/opt/trn_rl_repo", "/root/.axon_site/_ro/trn_rl_repo"):
    if os.path.isdir(_p) and _p not in sys.path:
        sys.path.append(_p)

import ml_dtypes

import concourse.bass as bass
import concourse.bacc as bacc
import concourse.mybir as mybir
import concourse.tile as tile
from concourse.bass_utils import run_bass_kernel_spmd

F32 = mybir.dt.float32
I32 = mybir.dt.int32
BF16 = mybir.dt.bfloat16
F8 = mybir.dt.float8e4
AF = mybir.ActivationFunctionType
ALU = mybir.AluOpType
DR = mybir.MatmulPerfMode.DoubleRow

B, C, H, W = 16, 512, 32, 32
HW = H * W                  # 1024
NCORES = 8
BPC = B // NCORES           # 2 batches per core
P = 128                     # SBUF partitions
CT = C // P                 # 4 channel tiles
JT = HW // P                # 8 key-position tiles
IBS = 512                   # i-block size (query positions per block)
IB = HW // IBS              # 2 i-blocks
GROUPS = 32
GSIZE = C // GROUPS         # 16 channels per group
EPS = 1e-6
SM_SCALE = float(int(C) ** -0.5)
EXP_BIAS = -3.0             # keeps exp() outputs well inside e4m3 range
MAGIC = 0x5F3759DF          # fp32 rsqrt seed

# prm rows: gn scales (x,y), gn biases (x,y), bq, bp'
R_SCALE, R_BIAS, R_BQ, R_BPP = 0, 2, 4, 5


def _emit(tc, aps):
    nc = tc.nc
    xs, ys, wq, wk, wv, wp, prm, amat, out = (
        aps["xs"], aps["ys"], aps["wqT"], aps["wkT"], aps["wvT"], aps["wpT"],
        aps["prm"], aps["amat"], aps["out"],
    )

    from contextlib import ExitStack

    with ExitStack() as ctx:
        cpool = ctx.enter_context(tc.tile_pool(name="const", bufs=1))
        wpool = ctx.enter_context(tc.tile_pool(name="w", bufs=1))
        xpool = ctx.enter_context(tc.tile_pool(name="xin", bufs=2))
        ypool = ctx.enter_context(tc.tile_pool(name="yin", bufs=2))
        hpool = ctx.enter_context(tc.tile_pool(name="hb", bufs=1))
        ynpool = ctx.enter_context(tc.tile_pool(name="yn", bufs=1))
        qpool = ctx.enter_context(tc.tile_pool(name="qb", bufs=1))
        kpool = ctx.enter_context(tc.tile_pool(name="kb", bufs=1))
        vpool = ctx.enter_context(tc.tile_pool(name="vb", bufs=1))
        epool = ctx.enter_context(tc.tile_pool(name="eb", bufs=1))
        opool = ctx.enter_context(tc.tile_pool(name="ob", bufs=2))
        rzpool = ctx.enter_context(tc.tile_pool(name="rz", bufs=2))
        outpool = ctx.enter_context(tc.tile_pool(name="outb", bufs=3))
        spool = ctx.enter_context(tc.tile_pool(name="small", bufs=2))
        pmm = ctx.enter_context(tc.tile_pool(name="pmm", bufs=3, space="PSUM"))
        pzb = ctx.enter_context(tc.tile_pool(name="pzb", bufs=1, space="PSUM"))
        pgs = ctx.enter_context(tc.tile_pool(name="pgs", bufs=1, space="PSUM"))

        def load_one(dst_sb, src, eng=None):
            v = src.rearrange("p (t n) -> p t n", n=HW)
            for t in range(CT):
                (eng or nc.sync).dma_start(dst_sb[:, t, :], v[:, t, :])

        def gn_stats_pre(src_sb, stats, half, uid):
            """DVE per-partition statistics into stats[:, half]: [mean, var,
            mean^2] per channel tile (mean^2 is filled later, combined)."""
            for t in range(CT):
                bns = spool.tile([P, 2, 6], F32, tag=f"bns{uid}")
                for h2 in range(2):
                    nc.vector.bn_stats(
                        bns[:, h2, :], src_sb[:, t, h2 * 512 : (h2 + 1) * 512]
                    )
                nc.vector.bn_aggr(stats[:, half, t, 0:2], bns[:])

        def gn_stats_post(stats, uid, halves=(0, 2), premixed=False):
            """Cross-partition group aggregation (one tiny PE matmul per half
            range) and the affine constants a, mb — all DVE, no ACT. halves
            selects a [lo, lo+n) slice of the x/y dim so the batch-0 x post
            can run before y statistics are even finished."""
            lo, n = halves[0], halves[1] - halves[0]
            sl = slice(lo, lo + n)
            if not premixed:
                nc.vector.tensor_tensor(
                    stats[:, sl, :, 2], stats[:, sl, :, 0], stats[:, sl, :, 0],
                    op=ALU.mult,
                )
            gps = pgs.tile([P, 2, CT, 3], F32, tag="gs")
            nc.tensor.matmul(gps[:, sl], amat_sb[:], stats[:, sl],
                             start=True, stop=True)
            g = spool.tile([P, 2, CT, 3], F32, tag=f"g{uid}")
            nc.vector.tensor_copy(g[:, sl], gps[:, sl])
            # var_g = E[var] + E[mean^2] - E[mean]^2  (equal-count partitions)
            msq = spool.tile([P, 2, CT], F32, tag=f"msq{uid}")
            nc.vector.tensor_tensor(msq[:, sl], g[:, sl, :, 0], g[:, sl, :, 0],
                                    op=ALU.mult)
            var = spool.tile([P, 2, CT], F32, tag=f"var{uid}")
            nc.vector.tensor_tensor(var[:, sl], g[:, sl, :, 1], g[:, sl, :, 2],
                                    op=ALU.add)
            nc.vector.tensor_tensor(var[:, sl], var[:, sl], msq[:, sl],
                                    op=ALU.subtract)
            nc.vector.tensor_scalar(var[:, sl], var[:, sl], EPS, None, op0=ALU.add)
            # rstd = rsqrt(var+eps): bit-trick seed + one Newton step gives
            # ~0.2% relative error, far below the fp8 noise floor
            seed = spool.tile([P, 2, CT], I32, tag=f"sd{uid}")
            nc.vector.tensor_scalar(
                seed[:, sl], var[:, sl].bitcast(I32), 1, None,
                op0=ALU.arith_shift_right
            )
            nc.vector.tensor_scalar(
                seed[:, sl], seed[:, sl], 0xFFFFFFFF, None, op0=ALU.bitwise_xor
            )
            nc.vector.tensor_scalar(
                seed[:, sl], seed[:, sl], MAGIC + 1, None, op0=ALU.add
            )
            r0 = seed[:, sl].bitcast(F32)
            t4 = spool.tile([P, 2, CT], F32, tag=f"t4{uid}")
            nc.vector.tensor_tensor(t4[:, sl], r0, r0, op=ALU.mult)
            nc.vector.tensor_tensor(t4[:, sl], t4[:, sl], var[:, sl], op=ALU.mult)
            nc.vector.tensor_scalar(t4[:, sl], t4[:, sl], -0.5, 1.5,
                                    op0=ALU.mult, op1=ALU.add)
            rstd = spool.tile([P, 2, CT], F32, tag=f"rs{uid}")
            nc.vector.tensor_tensor(rstd[:, sl], r0, t4[:, sl], op=ALU.mult)
            # a = rstd*gamma ; mb = beta - mean*a   (rows: scales 0:2, biases 2:4)
            a = spool.tile([P, 2, CT], F32, tag=f"a{uid}")
            nc.vector.tensor_tensor(
                a[:, sl], rstd[:, sl],
                prm_sb[:, R_SCALE + lo : R_SCALE + lo + n, :], op=ALU.mult
            )
            mb = spool.tile([P, 2, CT], F32, tag=f"mb{uid}")
            nc.vector.tensor_tensor(mb[:, sl], g[:, sl, :, 0], a[:, sl], op=ALU.mult)
            nc.vector.tensor_tensor(
                mb[:, sl], prm_sb[:, R_BIAS + lo : R_BIAS + lo + n, :], mb[:, sl],
                op=ALU.subtract
            )
            return a, mb

        def gn_apply(src_sb, dst_sb, st, half, engines=None):
            a, mb = st
            engs = engines or [nc.gpsimd] * CT
            for t in range(CT):
                if engs[t] is nc.scalar:
                    nc.scalar.activation(
                        dst_sb[:, t, :], src_sb[:, t, :], AF.Identity,
                        bias=mb[:, half, t : t + 1], scale=a[:, half, t : t + 1],
                    )
                else:
                    engs[t].tensor_scalar(
                        dst_sb[:, t, :], src_sb[:, t, :],
                        a[:, half, t : t + 1], mb[:, half, t : t + 1],
                        op0=ALU.mult, op1=ALU.add,
                    )

        # ---- prologue: x first (gates everything), then wk, y, rest ----
        x_sb = xpool.tile([P, CT, HW], BF16, tag="x")
        xv0 = xs[0].rearrange("p (t n) -> p t n", n=HW)
        for t, e in enumerate((nc.sync, nc.sync, nc.scalar, nc.scalar)):
            e.dma_start(x_sb[:, t, :], xv0[:, t, :])
        prm_sb = cpool.tile([P, 6, CT], F32)
        nc.sync.dma_start(prm_sb[:], prm.rearrange("p (q t) -> p q t", t=CT))
        amat_sb = cpool.tile([P, P], BF16)
        nc.sync.dma_start(amat_sb[:], amat[:])
        w_sb = {}

        def loadw(name, ap):
            t = wpool.tile([P, CT, C], F8, tag=name)
            nc.gpsimd.dma_start(t[:], ap.rearrange("p (t o) -> p t o", o=C))
            w_sb[name] = t

        loadw("wk", wk)
        y_sb = ypool.tile([P, CT, HW], BF16, tag="y")
        load_one(y_sb, ys[0], eng=nc.gpsimd)
        ones_mat = cpool.tile([P, 2, P], F8)
        nc.sync.dma_start(ones_mat[:], aps["ones"].rearrange("p (s q) -> p s q", s=2))
        ebias = cpool.tile([P, 1], F32)
        nc.vector.memset(ebias[:], EXP_BIAS)
        loadw("wv", wv)
        loadw("wq", wq)
        loadw("wp", wp)

        stats0 = spool.tile([P, 2, CT, 3], BF16, tag="st0")
        gn_stats_pre(x_sb, stats0, 0, uid="x0")
        st_x0 = gn_stats_post(stats0, uid="b0x", halves=(0, 1))
        gn_stats_pre(y_sb, stats0, 1, uid="y0")
        nxt = {}

        for b in range(BPC):
            outv = out[b].rearrange("p (t n) -> p t n", n=HW)

            xcur, ycur = x_sb, y_sb
            if nxt:
                h_sb, yn_sb = nxt.pop("h"), nxt.pop("yn")
            else:
                # batch 0 is latency-critical: apply h on DVE+ACT in parallel;
                # the y-side post/apply is deferred past the k/vT matmuls so
                # its tiny PE matmul never blocks them in the in-order queue
                h_sb = hpool.tile([P, CT, HW], F8, tag="h")
                gn_apply(xcur, h_sb, st_x0, 0,
                         engines=[nc.vector, nc.gpsimd, nc.vector, nc.gpsimd])
                yn_sb = None

            # ---- k = Wk h  (k[c_out, i]); one paired evac per mt on DVE ----
            k_sb = kpool.tile([P, CT, HW], F8, tag="k")
            for mt in range(CT):
                ps = pmm.tile([P, 2 * IBS], F32, tag="ps")
                for nh in range(IB):
                    for kp in range(0, CT, 2):
                        nc.tensor.matmul(
                            ps[:, nh * IBS : (nh + 1) * IBS],
                            w_sb["wk"][:, kp : kp + 2, mt * P : (mt + 1) * P],
                            h_sb[:, kp : kp + 2, nh * IBS : (nh + 1) * IBS],
                            start=(kp == 0), stop=(kp == CT - 2),
                            perf_mode=DR,
                        )
                nc.scalar.copy(k_sb[:, mt, :], ps[:])

            # ---- vT[j, c_out] = h^T WvT; paired evac per jt-pair on ACT ----
            vT_sb = vpool.tile([P, JT, C], F8, tag="vT")
            for jp in range(0, JT, 2):
                ps = pmm.tile([P, 2 * C], F32, tag="ps")
                for u in range(2):
                    for kp in range(0, CT, 2):
                        nc.tensor.matmul(
                            ps[:, u * C : (u + 1) * C],
                            h_sb[:, kp : kp + 2, (jp + u) * P : (jp + u + 1) * P],
                            w_sb["wv"][:, kp : kp + 2, :],
                            start=(kp == 0), stop=(kp == CT - 2),
                            perf_mode=DR,
                        )
                nc.scalar.copy(vT_sb[:, jp : jp + 2, :], ps[:])

            if yn_sb is None:
                # batch-0 y GroupNorm: post + apply now that k/vT are queued
                st_y0 = gn_stats_post(stats0, uid="b0y", halves=(1, 2))
                yn_sb = ynpool.tile([P, CT, HW], F8, tag="yn")
                gn_apply(ycur, yn_sb, st_y0, 1,
                         engines=[nc.gpsimd, nc.vector, nc.gpsimd, nc.vector])

            # ---- q = Wq y_ + bq; paired evac per mt on ACT (bias add) ----
            q_sb = qpool.tile([P, CT, HW], F8, tag="q")
            for mt in range(CT):
                ps = pmm.tile([P, 2 * IBS], F32, tag="ps")
                for nh in range(IB):
                    for kp in range(0, CT, 2):
                        nc.tensor.matmul(
                            ps[:, nh * IBS : (nh + 1) * IBS],
                            w_sb["wq"][:, kp : kp + 2, mt * P : (mt + 1) * P],
                            yn_sb[:, kp : kp + 2, nh * IBS : (nh + 1) * IBS],
                            start=(kp == 0), stop=(kp == CT - 2),
                            perf_mode=DR,
                        )
                nc.scalar.activation(
                    q_sb[:, mt, :], ps[:], AF.Identity,
                    bias=prm_sb[:, R_BQ, mt : mt + 1],
                )

            # prefetch next batch + its DVE-only stats; the tiny stats
            # matmul is deferred so it never blocks this batch's attention
            if b + 1 < BPC:
                xn_t = xpool.tile([P, CT, HW], BF16, tag="x")
                load_one(xn_t, xs[b + 1])
                yn_t = ypool.tile([P, CT, HW], BF16, tag="y")
                load_one(yn_t, ys[b + 1], eng=nc.gpsimd)
                statsn = spool.tile([P, 2, CT, 3], BF16, tag="st1")
                gn_stats_pre(xn_t, statsn, 0, uid=f"x{b+1}")
                gn_stats_pre(yn_t, statsn, 1, uid=f"y{b+1}")

            # ---- attention; r-projection pipelined one i-block behind ----
            rdefer = []

            def emit_r(ib, mps=(0, 2)):
                isl2 = slice(ib * IBS, (ib + 1) * IBS)
                o0_p = rdefer[0]
                for mp in mps:
                    ps = pmm.tile([P, 2 * IBS], F32, tag="ps")
                    for u in range(2):
                        for cp in range(0, CT, 2):
                            nc.tensor.matmul(
                                ps[:, u * IBS : (u + 1) * IBS],
                                w_sb["wp"][:, cp : cp + 2, (mp + u) * P : (mp + u + 1) * P],
                                o0_p[:, cp : cp + 2, :],
                                start=(cp == 0), stop=(cp == CT - 2),
                                perf_mode=DR,
                            )
                    for u in range(2):
                        ot = outpool.tile([P, IBS], BF16, tag="ot")
                        nc.vector.scalar_tensor_tensor(
                            ot[:], ps[:, u * IBS : (u + 1) * IBS],
                            prm_sb[:, R_BPP, mp + u : mp + u + 1],
                            xcur[:, mp + u, isl2], op0=ALU.add, op1=ALU.add,
                        )
                        nc.sync.dma_start(outv[:, mp + u, isl2], ot[:])

            for ib in range(IB):
                isl = slice(ib * IBS, (ib + 1) * IBS)
                e_sb = epool.tile([P, JT, IBS], F8, tag="e")
                zb = pzb.tile([P, IBS], F32, tag="zb")
                # S and exp per key-tile pair; Z (ones-matmul column sums of
                # E) lags one pair behind so the in-order PE never waits on
                # the ACT-engine exp.
                for jp in range(0, JT, 2):
                    ps = pmm.tile([P, 2 * IBS], F32, tag="ps")
                    for u in range(2):
                        for kp in range(0, CT, 2):
                            nc.tensor.matmul(
                                ps[:, u * IBS : (u + 1) * IBS],
                                k_sb[:, kp : kp + 2, (jp + u) * P : (jp + u + 1) * P],
                                q_sb[:, kp : kp + 2, isl],
                                start=(kp == 0), stop=(kp == CT - 2),
                                perf_mode=DR,
                            )
                    # E = exp(S / sqrt(c) - 3); logits are O(1), no max needed
                    nc.scalar.activation(e_sb[:, jp : jp + 2, :], ps[:], AF.Exp,
                                         bias=ebias[:], scale=SM_SCALE)
                    if jp >= 4:
                        jz = jp - 4
                        nc.tensor.matmul(
                            zb[:], ones_mat[:], e_sb[:, jz : jz + 2, :],
                            start=(jz == 0), stop=False,
                            perf_mode=DR,
                        )
                # r-projection of the previous i-block fills the PE gap
                # while the last exp pairs drain on ACT
                if ib > 0:
                    emit_r(ib - 1)
                    rdefer.pop(0)
                for jz in (JT - 4, JT - 2):
                    nc.tensor.matmul(
                        zb[:], ones_mat[:], e_sb[:, jz : jz + 2, :],
                        start=False, stop=(jz == JT - 2),
                        perf_mode=DR,
                    )
                rzb = rzpool.tile([P, IBS], F32, tag="rzb")
                nc.vector.reciprocal_approx_fast(rzb[:], zb[:])

                o0_sb = opool.tile([P, CT, IBS], F8, tag="o0")
                for cp in range(0, CT, 2):
                    ps = pmm.tile([P, 2 * IBS], F32, tag="ps")
                    for u in range(2):
                        for jp in range(0, JT, 2):
                            nc.tensor.matmul(
                                ps[:, u * IBS : (u + 1) * IBS],
                                vT_sb[:, jp : jp + 2, (cp + u) * P : (cp + u + 1) * P],
                                e_sb[:, jp : jp + 2, :],
                                start=(jp == 0), stop=(jp == JT - 2),
                                perf_mode=DR,
                            )
                    for u in range(2):
                        nc.vector.tensor_tensor(
                            o0_sb[:, cp + u, :], ps[:, u * IBS : (u + 1) * IBS],
                            rzb[:], op=ALU.mult,
                        )
                rdefer.append(o0_sb)
                # next batch's GroupNorm chain launches after the first
                # i-block so GpSimd has the whole second i-block to finish
                # h/yn before the next batch's projections need them
                if ib == 0 and b + 1 < BPC:
                    stn = gn_stats_post(statsn, uid=f"b{b+1}")
                    h_n = hpool.tile([P, CT, HW], F8, tag="h")
                    gn_apply(xn_t, h_n, stn, 0)
                    yn_n = ynpool.tile([P, CT, HW], F8, tag="yn")
                    gn_apply(yn_t, yn_n, stn, 1)
                    nxt = {"h": h_n, "yn": yn_n}
            emit_r(IB - 1)
            if b + 1 < BPC:
                x_sb, y_sb = xn_t, yn_t


_CACHE = {}


def _build():
    if "nc" in _CACHE:
        return _CACHE["nc"]
    nc = bacc.Bacc("TRN2", target_bir_lowering=False, debug=False)
    aps = {
        "xs": nc.dram_tensor("xs", [BPC, P, CT * HW], BF16, kind="ExternalInput").ap(),
        "ys": nc.dram_tensor("ys", [BPC, P, CT * HW], BF16, kind="ExternalInput").ap(),
        "wqT": nc.dram_tensor("wqT", [P, CT * C], F8, kind="ExternalInput").ap(),
        "wkT": nc.dram_tensor("wkT", [P, CT * C], F8, kind="ExternalInput").ap(),
        "wvT": nc.dram_tensor("wvT", [P, CT * C], F8, kind="ExternalInput").ap(),
        "wpT": nc.dram_tensor("wpT", [P, CT * C], F8, kind="ExternalInput").ap(),
        "prm": nc.dram_tensor("prm", [P, 6 * CT], F32, kind="ExternalInput").ap(),
        "amat": nc.dram_tensor("amat", [P, P], BF16, kind="ExternalInput").ap(),
        "ones": nc.dram_tensor("ones", [P, 2 * P], F8, kind="ExternalInput").ap(),
        "out": nc.dram_tensor("out", [BPC, P, CT * HW], BF16, kind="ExternalOutput").ap(),
    }
    with tile.TileContext(nc) as tc:
        _emit(tc, aps)
    nc.compile()
    _CACHE["nc"] = nc
    return nc


def _pack_chw(a, dtype):
    """[*, C, HW] -> [*, P, CT*HW] matching SBUF layout c = t*128 + p."""
    lead = a.shape[:-2]
    a = a.reshape(*lead, CT, P, HW)
    a = np.moveaxis(a, -3, -2)          # [..., P, CT, HW]
    return np.ascontiguousarray(
        a.reshape(*lead, P, CT * HW).astype(dtype)
    )


def _unpack_chw(a):
    """[*, P, CT*HW] -> [*, C, HW]."""
    a = np.asarray(a).astype(np.float32)
    lead = a.shape[:-2]
    a = a.reshape(*lead, P, CT, HW)
    a = np.moveaxis(a, -2, -3)          # [..., CT, P, HW]
    return np.ascontiguousarray(a.reshape(*lead, CT * P, HW))


def _host_inputs(x, y, norm_scale, norm_bias, norm1_scale, norm1_bias,
                 wq, bq, wk, bk, wv, bv, wp, bp):
    f = lambda a: np.ascontiguousarray(np.asarray(a, dtype=np.float32))
    x = f(x).reshape(B, C, HW)
    y = f(y).reshape(B, C, HW)
    wq, wk, wv, wp = f(wq), f(wk), f(wv), f(wp)
    # bk cancels in softmax; bv folds into bp' because softmax rows sum to 1
    bpp = f(bp) + wp @ f(bv)
    # rows: [gn_scale, gn1_scale, gn_bias, gn1_bias, bq, bp'] so the x/y
    # scale (and bias) pairs are adjacent for combined-stats processing
    prm = np.stack([f(norm_scale), f(norm1_scale), f(norm_bias), f(norm1_bias),
                    f(bq), bpp]).astype(np.float32)
    # [6, C] -> [P, 6*CT] matching prm_sb[p, q, t]
    prm = np.ascontiguousarray(
        prm.reshape(6, CT, P).transpose(2, 0, 1).reshape(P, 6 * CT)
    )
    amat = np.zeros((P, P), np.float32)
    for g in range(P // GSIZE):
        amat[g * GSIZE : (g + 1) * GSIZE, g * GSIZE : (g + 1) * GSIZE] = 1.0 / GSIZE

    def packw(w):
        # wT [c_in, c_out] -> [P, CT*C] matching w_sb[p, kt, o]; e4m3 with
        # clip to the TRN +-240 max (values beyond round to inf)
        wT8 = np.clip(w.T, -240.0, 240.0).astype(ml_dtypes.float8_e4m3)
        return np.ascontiguousarray(
            wT8.reshape(CT, P, C).transpose(1, 0, 2).reshape(P, CT * C)
        )

    shared = {
        "wqT": packw(wq), "wkT": packw(wk), "wvT": packw(wv), "wpT": packw(wp),
        "prm": prm, "amat": amat.astype(ml_dtypes.bfloat16),
        "ones": np.ones((P, 2 * P), ml_dtypes.float8_e4m3),
    }
    in_maps = []
    for core in range(NCORES):
        sl = slice(core * BPC, (core + 1) * BPC)
        in_maps.append({
            "xs": _pack_chw(x[sl], ml_dtypes.bfloat16),
            "ys": _pack_chw(y[sl], ml_dtypes.bfloat16),
            **shared,
        })
    return in_maps


def _run(in_maps, trace=False):
    nc = _build()
    res = run_bass_kernel_spmd(
        nc, in_maps, core_ids=list(range(NCORES)), trace=trace
    )
    out = np.concatenate(
        [_unpack_chw(res.results[i]["out"]) for i in range(NCORES)], axis=0
    ).reshape(B, C, H, W)
    return out, res


def kernel(**inputs):
    in_maps = _host_inputs(**inputs)
    out, _ = _run(in_maps, trace=False)
    return out


# revision 47
# speedup vs baseline: 1.2414x; 1.0004x over previous
"""AttnBlock fusion kernel for Trainium2 (Bass/Tile), 8 NeuronCores.

Reference computation (per batch element b; c=512 channels, hw=1024 spatial):
    h  = GroupNorm(32, c)(x) ; k = Wk h + bk ; v = Wv h + bv
    y_ = GroupNorm(32, c)(y) ; q = Wq y_ + bq
    attn = softmax_j(q^T k / sqrt(c)) ; o = v @ attn^T ; out = x + Wp o + bp

Sharding: pure data parallel over batch (16 batches / 8 cores = 2 each).

Key algebraic moves (exactness notes inline):
  * bk cancels in softmax over j (adds a per-i constant to logits) -> dropped.
  * bv contributes bv * sum_j(attn) = bv exactly -> folded into
    bp' = bp + Wp @ bv on the host.
  * v is produced directly transposed (vT[j,c]) by the projection matmul,
    and attention scores are computed as S[j,i]; no on-chip transposes.
  * softmax denominator: ones matmul gives the partition-broadcast
    column sum of exp(S) directly in PSUM.

Precision strategy (tolerance is rel_l2 < 2e-2; this lands ~7e-3):
  * x/y stream in as bf16 (halves input DMA); GroupNorm statistics and all
    PSUM accumulation stay fp32.
  * All matmuls run in fp8 e4m3 with perf_mode=DoubleRow (2 fp8 weights
    per PE cell -> 2 contraction rows/instruction). Operands are laid out
    [P, ksub, free] so a pair of 128-row k-subtiles feeds each instruction.
  * exp uses bias=-3 so e^(logit-3) stays far below the e4m3 +-240
    limit (TRN fp8e4 overflows to inf, which would poison the matmul).
    The -3 cancels exactly in softmax normalization.
  * Output is written bf16 and upcast on host.

Engine budget notes: DVE/ACT instructions cost ~0.3us fixed overhead each,
so PSUM results are paired into [P, 2, 512] tiles (two banks) and
evacuated with a single op. GroupNorm statistics post-processing runs
entirely on DVE (bit-trick rsqrt) so the ACT table never leaves Exp
during attention; GroupNorm application runs on GpSimd (SBUF-only).
"""

import math
import os
import sys

import numpy as np

for _p in ("/opt/trn_rl_repo", "/root/.axon_site/_ro/trn_rl_repo"):
    if os.path.isdir(_p) and _p not in sys.path:
        sys.path.append(_p)

import ml_dtypes

import concourse.bass as bass
import concourse.bacc as bacc
import concourse.mybir as mybir
import concourse.tile as tile
from concourse.bass_utils import run_bass_kernel_spmd

F32 = mybir.dt.float32
I32 = mybir.dt.int32
BF16 = mybir.dt.bfloat16
F8 = mybir.dt.float8e4
AF = mybir.ActivationFunctionType
ALU = mybir.AluOpType
DR = mybir.MatmulPerfMode.DoubleRow

B, C, H, W = 16, 512, 32, 32
HW = H * W                  # 1024
NCORES = 8
BPC = B // NCORES           # 2 batches per core
P = 128                     # SBUF partitions
CT = C // P                 # 4 channel tiles
JT = HW // P                # 8 key-position tiles
IBS = 512                   # i-block size (query positions per block)
IB = HW // IBS              # 2 i-blocks
GROUPS = 32
GSIZE = C // GROUPS         # 16 channels per group
EPS = 1e-6
SM_SCALE = float(int(C) ** -0.5)
EXP_BIAS = -3.0             # keeps exp() outputs well inside e4m3 range
MAGIC = 0x5F3759DF          # fp32 rsqrt seed

# prm rows: gn scales (x,y), gn biases (x,y), bq, bp'
R_SCALE, R_BIAS, R_BQ, R_BPP = 0, 2, 4, 5


def _emit(tc, aps):
    nc = tc.nc
    xs, ys, wq, wk, wv, wp, prm, amat, out = (
        aps["xs"], aps["ys"], aps["wqT"], aps["wkT"], aps["wvT"], aps["wpT"],
        aps["prm"], aps["amat"], aps["out"],
    )

    from contextlib import ExitStack

    with ExitStack() as ctx:
        cpool = ctx.enter_context(tc.tile_pool(name="const", bufs=1))
        wpool = ctx.enter_context(tc.tile_pool(name="w", bufs=1))
        xpool = ctx.enter_context(tc.tile_pool(name="xin", bufs=2))
        ypool = ctx.enter_context(tc.tile_pool(name="yin", bufs=2))
        hpool = ctx.enter_context(tc.tile_pool(name="hb", bufs=1))
        ynpool = ctx.enter_context(tc.tile_pool(name="yn", bufs=1))
        qpool = ctx.enter_context(tc.tile_pool(name="qb", bufs=1))
        kpool = ctx.enter_context(tc.tile_pool(name="kb", bufs=1))
        vpool = ctx.enter_context(tc.tile_pool(name="vb", bufs=1))
        epool = ctx.enter_context(tc.tile_pool(name="eb", bufs=1))
        opool = ctx.enter_context(tc.tile_pool(name="ob", bufs=3))
        rzpool = ctx.enter_context(tc.tile_pool(name="rz", bufs=2))
        outpool = ctx.enter_context(tc.tile_pool(name="outb", bufs=3))
        spool = ctx.enter_context(tc.tile_pool(name="small", bufs=2))
        pmm = ctx.enter_context(tc.tile_pool(name="pmm", bufs=3, space="PSUM"))
        pzb = ctx.enter_context(tc.tile_pool(name="pzb", bufs=1, space="PSUM"))
        pgs = ctx.enter_context(tc.tile_pool(name="pgs", bufs=1, space="PSUM"))

        def load_one(dst_sb, src, eng=None):
            v = src.rearrange("p (t n) -> p t n", n=HW)
            for t in range(CT):
                (eng or nc.sync).dma_start(dst_sb[:, t, :], v[:, t, :])

        def gn_stats_pre(src_sb, stats, half, uid):
            """DVE per-partition statistics into stats[:, half]: [mean, var,
            mean^2] per channel tile (mean^2 is filled later, combined)."""
            for t in range(CT):
                bns = spool.tile([P, 2, 6], F32, tag=f"bns{uid}")
                for h2 in range(2):
                    nc.vector.bn_stats(
                        bns[:, h2, :], src_sb[:, t, h2 * 512 : (h2 + 1) * 512]
                    )
                nc.vector.bn_aggr(stats[:, half, t, 0:2], bns[:])

        def gn_stats_post(stats, uid, halves=(0, 2), premixed=False):
            """Cross-partition group aggregation (one tiny PE matmul per half
            range) and the affine constants a, mb — all DVE, no ACT. halves
            selects a [lo, lo+n) slice of the x/y dim so the batch-0 x post
            can run before y statistics are even finished."""
            lo, n = halves[0], halves[1] - halves[0]
            sl = slice(lo, lo + n)
            if not premixed:
                nc.vector.tensor_tensor(
                    stats[:, sl, :, 2], stats[:, sl, :, 0], stats[:, sl, :, 0],
                    op=ALU.mult,
                )
            gps = pgs.tile([P, 2, CT, 3], F32, tag="gs")
            nc.tensor.matmul(gps[:, sl], amat_sb[:], stats[:, sl],
                             start=True, stop=True)
            g = spool.tile([P, 2, CT, 3], F32, tag=f"g{uid}")
            nc.vector.tensor_copy(g[:, sl], gps[:, sl])
            # var_g = E[var] + E[mean^2] - E[mean]^2  (equal-count partitions)
            msq = spool.tile([P, 2, CT], F32, tag=f"msq{uid}")
            nc.vector.tensor_tensor(msq[:, sl], g[:, sl, :, 0], g[:, sl, :, 0],
                                    op=ALU.mult)
            var = spool.tile([P, 2, CT], F32, tag=f"var{uid}")
            nc.vector.tensor_tensor(var[:, sl], g[:, sl, :, 1], g[:, sl, :, 2],
                                    op=ALU.add)
            nc.vector.tensor_tensor(var[:, sl], var[:, sl], msq[:, sl],
                                    op=ALU.subtract)
            nc.vector.tensor_scalar(var[:, sl], var[:, sl], EPS, None, op0=ALU.add)
            # rstd = rsqrt(var+eps): bit-trick seed + one Newton step gives
            # ~0.2% relative error, far below the fp8 noise floor
            seed = spool.tile([P, 2, CT], I32, tag=f"sd{uid}")
            nc.vector.tensor_scalar(
                seed[:, sl], var[:, sl].bitcast(I32), 1, None,
                op0=ALU.arith_shift_right
            )
            nc.vector.tensor_scalar(
                seed[:, sl], seed[:, sl], 0xFFFFFFFF, None, op0=ALU.bitwise_xor
            )
            nc.vector.tensor_scalar(
                seed[:, sl], seed[:, sl], MAGIC + 1, None, op0=ALU.add
            )
            r0 = seed[:, sl].bitcast(F32)
            t4 = spool.tile([P, 2, CT], F32, tag=f"t4{uid}")
            nc.vector.tensor_tensor(t4[:, sl], r0, r0, op=ALU.mult)
            nc.vector.tensor_tensor(t4[:, sl], t4[:, sl], var[:, sl], op=ALU.mult)
            nc.vector.tensor_scalar(t4[:, sl], t4[:, sl], -0.5, 1.5,
                                    op0=ALU.mult, op1=ALU.add)
            rstd = spool.tile([P, 2, CT], F32, tag=f"rs{uid}")
            nc.vector.tensor_tensor(rstd[:, sl], r0, t4[:, sl], op=ALU.mult)
            # a = rstd*gamma ; mb = beta - mean*a   (rows: scales 0:2, biases 2:4)
            a = spool.tile([P, 2, CT], F32, tag=f"a{uid}")
            nc.vector.tensor_tensor(
                a[:, sl], rstd[:, sl],
                prm_sb[:, R_SCALE + lo : R_SCALE + lo + n, :], op=ALU.mult
            )
            mb = spool.tile([P, 2, CT], F32, tag=f"mb{uid}")
            nc.vector.tensor_tensor(mb[:, sl], g[:, sl, :, 0], a[:, sl], op=ALU.mult)
            nc.vector.tensor_tensor(
                mb[:, sl], prm_sb[:, R_BIAS + lo : R_BIAS + lo + n, :], mb[:, sl],
                op=ALU.subtract
            )
            return a, mb

        def gn_apply(src_sb, dst_sb, st, half, engines=None):
            a, mb = st
            engs = engines or [nc.gpsimd] * CT
            for t in range(CT):
                if engs[t] is nc.scalar:
                    nc.scalar.activation(
                        dst_sb[:, t, :], src_sb[:, t, :], AF.Identity,
                        bias=mb[:, half, t : t + 1], scale=a[:, half, t : t + 1],
                    )
                else:
                    engs[t].tensor_scalar(
                        dst_sb[:, t, :], src_sb[:, t, :],
                        a[:, half, t : t + 1], mb[:, half, t : t + 1],
                        op0=ALU.mult, op1=ALU.add,
                    )

        # ---- prologue: x first (gates everything), then wk, y, rest ----
        x_sb = xpool.tile([P, CT, HW], BF16, tag="x")
        xv0 = xs[0].rearrange("p (t n) -> p t n", n=HW)
        for t, e in enumerate((nc.sync, nc.sync, nc.scalar, nc.scalar)):
            e.dma_start(x_sb[:, t, :], xv0[:, t, :])
        prm_sb = cpool.tile([P, 6, CT], F32)
        nc.sync.dma_start(prm_sb[:], prm.rearrange("p (q t) -> p q t", t=CT))
        amat_sb = cpool.tile([P, P], BF16)
        nc.sync.dma_start(amat_sb[:], amat[:])
        w_sb = {}

        def loadw(name, ap):
            t = wpool.tile([P, CT, C], F8, tag=name)
            nc.gpsimd.dma_start(t[:], ap.rearrange("p (t o) -> p t o", o=C))
            w_sb[name] = t

        loadw("wk", wk)
        ones_mat = cpool.tile([P, 2, P], F8)
        nc.sync.dma_start(ones_mat[:], aps["ones"].rearrange("p (s q) -> p s q", s=2))
        ebias = cpool.tile([P, 1], F32)
        nc.vector.memset(ebias[:], EXP_BIAS)
        loadw("wv", wv)
        loadw("wq", wq)
        loadw("wp", wp)
        # y lands after the weights so its DVE statistics can't interleave
        # with (and stretch) the critical batch-0 x post chain
        y_sb = ypool.tile([P, CT, HW], BF16, tag="y")
        load_one(y_sb, ys[0], eng=nc.gpsimd)

        stats0 = spool.tile([P, 2, CT, 3], BF16, tag="st0")
        gn_stats_pre(x_sb, stats0, 0, uid="x0")
        st_x0 = gn_stats_post(stats0, uid="b0x", halves=(0, 1))
        gn_stats_pre(y_sb, stats0, 1, uid="y0")
        nxt = {}

        for b in range(BPC):
            outv = out[b].rearrange("p (t n) -> p t n", n=HW)

            xcur, ycur = x_sb, y_sb
            if nxt:
                h_sb, yn_sb = nxt.pop("h"), nxt.pop("yn")
            else:
                # batch 0 is latency-critical: apply h on DVE+ACT in parallel;
                # the y-side post/apply is deferred past the k/vT matmuls so
                # its tiny PE matmul never blocks them in the in-order queue
                h_sb = hpool.tile([P, CT, HW], F8, tag="h")
                gn_apply(xcur, h_sb, st_x0, 0,
                         engines=[nc.gpsimd, nc.scalar, nc.gpsimd, nc.scalar])
                yn_sb = None

            # ---- k = Wk h  (k[c_out, i]); one paired evac per mt on DVE ----
            k_sb = kpool.tile([P, CT, HW], F8, tag="k")
            for mt in range(CT):
                ps = pmm.tile([P, 2 * IBS], F32, tag="ps")
                for nh in range(IB):
                    for kp in range(0, CT, 2):
                        nc.tensor.matmul(
                            ps[:, nh * IBS : (nh + 1) * IBS],
                            w_sb["wk"][:, kp : kp + 2, mt * P : (mt + 1) * P],
                            h_sb[:, kp : kp + 2, nh * IBS : (nh + 1) * IBS],
                            start=(kp == 0), stop=(kp == CT - 2),
                            perf_mode=DR,
                        )
                if mt % 2 == 0:
                    nc.scalar.copy(k_sb[:, mt, :], ps[:])
                else:
                    nc.vector.tensor_copy(k_sb[:, mt, :], ps[:])

            # ---- vT[j, c_out] = h^T WvT; paired evac per jt-pair on ACT ----
            vT_sb = vpool.tile([P, JT, C], F8, tag="vT")
            for jp in range(0, JT, 2):
                ps = pmm.tile([P, 2 * C], F32, tag="ps")
                for u in range(2):
                    for kp in range(0, CT, 2):
                        nc.tensor.matmul(
                            ps[:, u * C : (u + 1) * C],
                            h_sb[:, kp : kp + 2, (jp + u) * P : (jp + u + 1) * P],
                            w_sb["wv"][:, kp : kp + 2, :],
                            start=(kp == 0), stop=(kp == CT - 2),
                            perf_mode=DR,
                        )
                nc.scalar.copy(vT_sb[:, jp : jp + 2, :], ps[:])

            if yn_sb is None:
                # batch-0 y GroupNorm: post + apply now that k/vT are queued
                st_y0 = gn_stats_post(stats0, uid="b0y", halves=(1, 2))
                yn_sb = ynpool.tile([P, CT, HW], F8, tag="yn")
                gn_apply(ycur, yn_sb, st_y0, 1,
                         engines=[nc.gpsimd, nc.vector, nc.gpsimd, nc.vector])

            # ---- q = Wq y_ + bq; paired evac per mt on ACT (bias add) ----
            q_sb = qpool.tile([P, CT, HW], F8, tag="q")
            for mt in range(CT):
                ps = pmm.tile([P, 2 * IBS], F32, tag="ps")
                for nh in range(IB):
                    for kp in range(0, CT, 2):
                        nc.tensor.matmul(
                            ps[:, nh * IBS : (nh + 1) * IBS],
                            w_sb["wq"][:, kp : kp + 2, mt * P : (mt + 1) * P],
                            yn_sb[:, kp : kp + 2, nh * IBS : (nh + 1) * IBS],
                            start=(kp == 0), stop=(kp == CT - 2),
                            perf_mode=DR,
                        )
                if mt % 2 == 0:
                    nc.scalar.activation(
                        q_sb[:, mt, :], ps[:], AF.Identity,
                        bias=prm_sb[:, R_BQ, mt : mt + 1],
                    )
                else:
                    nc.vector.tensor_scalar(
                        q_sb[:, mt, :], ps[:],
                        prm_sb[:, R_BQ, mt : mt + 1], None, op0=ALU.add,
                    )

            # prefetch next batch + its DVE-only stats; the tiny stats
            # matmul is deferred so it never blocks this batch's attention
            if b + 1 < BPC:
                xn_t = xpool.tile([P, CT, HW], BF16, tag="x")
                load_one(xn_t, xs[b + 1])
                yn_t = ypool.tile([P, CT, HW], BF16, tag="y")
                load_one(yn_t, ys[b + 1], eng=nc.gpsimd)
                statsn = spool.tile([P, 2, CT, 3], BF16, tag="st1")
                gn_stats_pre(xn_t, statsn, 0, uid=f"x{b+1}")
                gn_stats_pre(yn_t, statsn, 1, uid=f"y{b+1}")

            # ---- attention; r-projection pipelined one i-block behind ----
            rdefer = []

            def emit_r(ib, mps=(0, 2)):
                isl2 = slice(ib * IBS, (ib + 1) * IBS)
                o0_p = rdefer[0]
                for mp in mps:
                    ps = pmm.tile([P, 2 * IBS], F32, tag="ps")
                    for u in range(2):
                        for cp in range(0, CT, 2):
                            nc.tensor.matmul(
                                ps[:, u * IBS : (u + 1) * IBS],
                                w_sb["wp"][:, cp : cp + 2, (mp + u) * P : (mp + u + 1) * P],
                                o0_p[:, cp : cp + 2, :],
                                start=(cp == 0), stop=(cp == CT - 2),
                                perf_mode=DR,
                            )
                    for u in range(2):
                        ot = outpool.tile([P, IBS], BF16, tag="ot")
                        nc.vector.scalar_tensor_tensor(
                            ot[:], ps[:, u * IBS : (u + 1) * IBS],
                            prm_sb[:, R_BPP, mp + u : mp + u + 1],
                            xcur[:, mp + u, isl2], op0=ALU.add, op1=ALU.add,
                        )
                        nc.sync.dma_start(outv[:, mp + u, isl2], ot[:])

            for ib in range(IB):
                isl = slice(ib * IBS, (ib + 1) * IBS)
                e_sb = epool.tile([P, JT, IBS], F8, tag="e")
                zb = pzb.tile([P, IBS], F32, tag="zb")
                # S and exp per key-tile pair; Z (ones-matmul column sums of
                # E) lags one pair behind so the in-order PE never waits on
                # the ACT-engine exp.
                for jp in range(0, JT, 2):
                    ps = pmm.tile([P, 2 * IBS], F32, tag="ps")
                    for u in range(2):
                        for kp in range(0, CT, 2):
                            nc.tensor.matmul(
                                ps[:, u * IBS : (u + 1) * IBS],
                                k_sb[:, kp : kp + 2, (jp + u) * P : (jp + u + 1) * P],
                                q_sb[:, kp : kp + 2, isl],
                                start=(kp == 0), stop=(kp == CT - 2),
                                perf_mode=DR,
                            )
                    # E = exp(S / sqrt(c) - 3); logits are O(1), no max needed
                    nc.scalar.activation(e_sb[:, jp : jp + 2, :], ps[:], AF.Exp,
                                         bias=ebias[:], scale=SM_SCALE)
                    if jp >= 4:
                        jz = jp - 4
                        nc.tensor.matmul(
                            zb[:], ones_mat[:], e_sb[:, jz : jz + 2, :],
                            start=(jz == 0), stop=False,
                            perf_mode=DR,
                        )
                # r-projection of the previous i-block fills the PE gap
                # while the last exp pairs drain on ACT
                if ib > 0:
                    emit_r(ib - 1)
                    rdefer.pop(0)
                for jz in (JT - 4, JT - 2):
                    nc.tensor.matmul(
                        zb[:], ones_mat[:], e_sb[:, jz : jz + 2, :],
                        start=False, stop=(jz == JT - 2),
                        perf_mode=DR,
                    )
                rzb = rzpool.tile([P, IBS], F32, tag="rzb")
                nc.vector.reciprocal_approx_fast(rzb[:], zb[:])

                o0_sb = opool.tile([P, CT, IBS], F8, tag="o0")
                for cp in range(0, CT, 2):
                    ps = pmm.tile([P, 2 * IBS], F32, tag="ps")
                    for u in range(2):
                        for jp in range(0, JT, 2):
                            nc.tensor.matmul(
                                ps[:, u * IBS : (u + 1) * IBS],
                                vT_sb[:, jp : jp + 2, (cp + u) * P : (cp + u + 1) * P],
                                e_sb[:, jp : jp + 2, :],
                                start=(jp == 0), stop=(jp == JT - 2),
                                perf_mode=DR,
                            )
                    for u in range(2):
                        nc.vector.tensor_tensor(
                            o0_sb[:, cp + u, :], ps[:, u * IBS : (u + 1) * IBS],
                            rzb[:], op=ALU.mult,
                        )
                rdefer.append(o0_sb)
                # next batch's GroupNorm chain launches after the first
                # i-block so GpSimd has the whole second i-block to finish
                # h/yn before the next batch's projections need them
                if ib == 0 and b + 1 < BPC:
                    stn = gn_stats_post(statsn, uid=f"b{b+1}")
                    h_n = hpool.tile([P, CT, HW], F8, tag="h")
                    gn_apply(xn_t, h_n, stn, 0)
                    yn_n = ynpool.tile([P, CT, HW], F8, tag="yn")
                    gn_apply(yn_t, yn_n, stn, 1)
                    nxt = {"h": h_n, "yn": yn_n}
            emit_r(IB - 1)
            if b + 1 < BPC:
                x_sb, y_sb = xn_t, yn_t


_CACHE = {}


def _build():
    if "nc" in _CACHE:
        return _CACHE["nc"]
    nc = bacc.Bacc("TRN2", target_bir_lowering=False, debug=False)
    aps = {
        "xs": nc.dram_tensor("xs", [BPC, P, CT * HW], BF16, kind="ExternalInput").ap(),
        "ys": nc.dram_tensor("ys", [BPC, P, CT * HW], BF16, kind="ExternalInput").ap(),
        "wqT": nc.dram_tensor("wqT", [P, CT * C], F8, kind="ExternalInput").ap(),
        "wkT": nc.dram_tensor("wkT", [P, CT * C], F8, kind="ExternalInput").ap(),
        "wvT": nc.dram_tensor("wvT", [P, CT * C], F8, kind="ExternalInput").ap(),
        "wpT": nc.dram_tensor("wpT", [P, CT * C], F8, kind="ExternalInput").ap(),
        "prm": nc.dram_tensor("prm", [P, 6 * CT], F32, kind="ExternalInput").ap(),
        "amat": nc.dram_tensor("amat", [P, P], BF16, kind="ExternalInput").ap(),
        "ones": nc.dram_tensor("ones", [P, 2 * P], F8, kind="ExternalInput").ap(),
        "out": nc.dram_tensor("out", [BPC, P, CT * HW], BF16, kind="ExternalOutput").ap(),
    }
    with tile.TileContext(nc) as tc:
        _emit(tc, aps)
    nc.compile()
    _CACHE["nc"] = nc
    return nc


def _pack_chw(a, dtype):
    """[*, C, HW] -> [*, P, CT*HW] matching SBUF layout c = t*128 + p."""
    lead = a.shape[:-2]
    a = a.reshape(*lead, CT, P, HW)
    a = np.moveaxis(a, -3, -2)          # [..., P, CT, HW]
    return np.ascontiguousarray(
        a.reshape(*lead, P, CT * HW).astype(dtype)
    )


def _unpack_chw(a):
    """[*, P, CT*HW] -> [*, C, HW]."""
    a = np.asarray(a).astype(np.float32)
    lead = a.shape[:-2]
    a = a.reshape(*lead, P, CT, HW)
    a = np.moveaxis(a, -2, -3)          # [..., CT, P, HW]
    return np.ascontiguousarray(a.reshape(*lead, CT * P, HW))


def _host_inputs(x, y, norm_scale, norm_bias, norm1_scale, norm1_bias,
                 wq, bq, wk, bk, wv, bv, wp, bp):
    f = lambda a: np.ascontiguousarray(np.asarray(a, dtype=np.float32))
    x = f(x).reshape(B, C, HW)
    y = f(y).reshape(B, C, HW)
    wq, wk, wv, wp = f(wq), f(wk), f(wv), f(wp)
    # bk cancels in softmax; bv folds into bp' because softmax rows sum to 1
    bpp = f(bp) + wp @ f(bv)
    # rows: [gn_scale, gn1_scale, gn_bias, gn1_bias, bq, bp'] so the x/y
    # scale (and bias) pairs are adjacent for combined-stats processing
    prm = np.stack([f(norm_scale), f(norm1_scale), f(norm_bias), f(norm1_bias),
                    f(bq), bpp]).astype(np.float32)
    # [6, C] -> [P, 6*CT] matching prm_sb[p, q, t]
    prm = np.ascontiguousarray(
        prm.reshape(6, CT, P).transpose(2, 0, 1).reshape(P, 6 * CT)
    )
    amat = np.zeros((P, P), np.float32)
    for g in range(P // GSIZE):
        amat[g * GSIZE : (g + 1) * GSIZE, g * GSIZE : (g + 1) * GSIZE] = 1.0 / GSIZE

    def packw(w):
        # wT [c_in, c_out] -> [P, CT*C] matching w_sb[p, kt, o]; e4m3 with
        # clip to the TRN +-240 max (values beyond round to inf)
        wT8 = np.clip(w.T, -240.0, 240.0).astype(ml_dtypes.float8_e4m3)
        return np.ascontiguousarray(
            wT8.reshape(CT, P, C).transpose(1, 0, 2).reshape(P, CT * C)
        )

    shared = {
        "wqT": packw(wq), "wkT": packw(wk), "wvT": packw(wv), "wpT": packw(wp),
        "prm": prm, "amat": amat.astype(ml_dtypes.bfloat16),
        "ones": np.ones((P, 2 * P), ml_dtypes.float8_e4m3),
    }
    in_maps = []
    for core in range(NCORES):
        sl = slice(core * BPC, (core + 1) * BPC)
        in_maps.append({
            "xs": _pack_chw(x[sl], ml_dtypes.bfloat16),
            "ys": _pack_chw(y[sl], ml_dtypes.bfloat16),
            **shared,
        })
    return in_maps


def _run(in_maps, trace=False):
    nc = _build()
    res = run_bass_kernel_spmd(
        nc, in_maps, core_ids=list(range(NCORES)), trace=trace
    )
    out = np.concatenate(
        [_unpack_chw(res.results[i]["out"]) for i in range(NCORES)], axis=0
    ).reshape(B, C, H, W)
    return out, res


def kernel(**inputs):
    in_maps = _host_inputs(**inputs)
    out, _ = _run(in_maps, trace=False)
    return out
